# revision 1
# baseline (speedup 1.0000x reference)
"""Trainium2 Bass kernel for nn_AdaptBlockV2 (deformable-conv-v2 block).

Data-parallel over the batch axis: 8 samples -> 8 NeuronCores, one sample
per core. Inside each core:
  A) load x; build zero-padded CHW copy (bf16) for the convs; transpose x to
     HWC and write a "quad" gather table to DRAM (row r = channels of flat
     pixels [r, r+1, r+W, r+W+1], bf16) -- one indirect-DMA descriptor then
     fetches all 4 bilinear corners of one (pixel, tap).
  B) 15-channel 3x3 conv (offset transform T, translation tr, modulation
     mask) as 9 PSUM-accumulated matmuls; transpose conv output to
     pixel-major; bulk DVE math for sampling positions py/px, floor via
     floored-mod, corner weights (bilinear x mask x validity), and the flat
     gather index.
  C) per-slice pipeline: indirect DMA gather -> DVE weighted 4-corner
     combine -> PE transpose of samp to (tap,channel)-major -> matmul with
     dw -> BN (running stats) + residual + ReLU epilogue -> DMA out.

The SWDGE gather descriptor generation on the Pool engine (~8ns/descriptor
x 62208 descriptors ~= 490us) is the hard floor; everything else is
banded/pipelined underneath it: the quad table is written in row bands (only
the ~147 rows that can be fetched with zero weight but stale memory are
zeroed), the offset math is grouped so the first gather issues ~15us in, and
each gather waits only on the table bands its slice can touch.

kernel(**inputs) takes FULL unsharded inputs, returns the FULL output.
"""
import numpy as np
import ml_dtypes

N, C, H, W = 8, 48, 96, 72
HW = H * W                       # 6912
LEAD = W + 2                     # 74: lead pad rows in the quad table
RQ = 7040                        # quad-table rows (55*128; >= HW+W+2)
QW = 256                         # quad-table row width (512B, dma_gather)
NB = HW // 128                   # 54 pixel blocks
QTOT = NB * 9                    # 486 (block, tap) chunks
PADW = W + 2                     # 74 padded conv row stride
PADLEN = (H + 2) * PADW         # 7252
BN_EPS = 1e-5
CONV_ROWS = 7                    # conv N-tile = 7 image rows = 504 pixels
SLICE_BLOCKS = 4                 # gather/combine slice = 4 pixel blocks
MAXD = 10                        # sample-displacement bound for slab deps

_REG = np.array([[-1, -1, -1, 0, 0, 0, 1, 1, 1],
                 [-1, 0, 1, -1, 0, 1, -1, 0, 1]], dtype=np.float32)

_built = {}


def _slices():
    out = []
    b = 0
    while b < NB:
        nb = min(SLICE_BLOCKS, NB - b)
        out.append((b, nb))
        b += nb
    return out


def build_nc():
    import concourse.bass as bass
    import concourse.bacc as bacc
    import concourse.tile as tile
    from concourse import mybir
    from concourse.bass import AP
    from concourse.masks import make_identity
    from concourse.tile import add_dep_helper
    from contextlib import ExitStack

    dt = mybir.dt
    op = mybir.AluOpType
    act = mybir.ActivationFunctionType

    nc = bacc.Bacc("TRN2", target_bir_lowering=False, debug=False,
                   num_devices=N, dynamic_dma_scratch_size=16384)
    x_ext = nc.declare_dram_parameter("x", [C, HW], dt.float32, isOutput=False)
    wconv_ext = nc.declare_dram_parameter("wconv", [C, 135], dt.bfloat16, isOutput=False)
    bconv_ext = nc.declare_dram_parameter("bconv", [15], dt.float32, isOutput=False)
    dwt_ext = nc.declare_dram_parameter("dwt", [128, 192], dt.bfloat16, isOutput=False)
    reg0_ext = nc.declare_dram_parameter("reg0", [128, 9], dt.float32, isOutput=False)
    reg1_ext = nc.declare_dram_parameter("reg1", [128, 9], dt.float32, isOutput=False)
    yc_ext = nc.declare_dram_parameter("ycoord", [128, NB], dt.float32, isOutput=False)
    xc_ext = nc.declare_dram_parameter("xcoord", [128, NB], dt.float32, isOutput=False)
    gamma_ext = nc.declare_dram_parameter("gamma", [C], dt.float32, isOutput=False)
    beta_ext = nc.declare_dram_parameter("beta", [C], dt.float32, isOutput=False)
    rmean_ext = nc.declare_dram_parameter("rmean", [C], dt.float32, isOutput=False)
    rvar_ext = nc.declare_dram_parameter("rvar", [C], dt.float32, isOutput=False)
    out_ext = nc.declare_dram_parameter("out", [C, HW], dt.float32, isOutput=True)

    x_quad = nc.dram_tensor("x_quad", [RQ, QW], dt.bfloat16)
    idx_dram = nc.dram_tensor("idx_dram", [128 * QTOT], dt.int16)

    with tile.TileContext(nc) as tc, ExitStack() as ctx:
        cp = ctx.enter_context(tc.tile_pool(name="const", bufs=1))
        tp = ctx.enter_context(tc.tile_pool(name="tmp", bufs=1))
        wp = ctx.enter_context(tc.tile_pool(name="work", bufs=2))
        pp_a = ctx.enter_context(tc.tile_pool(name="ps_a", bufs=2, space="PSUM"))
        pp_st = ctx.enter_context(tc.tile_pool(name="ps_st", bufs=2, space="PSUM"))
        pp_out = ctx.enter_context(tc.tile_pool(name="ps_out", bufs=1, space="PSUM"))

        # ---------------- constants / weights to SBUF ----------------
        x_sb = cp.tile([C, HW], dt.float32, tag="x_sb")
        nc.sync.dma_start(x_sb[:], x_ext[:])
        wconv_t = cp.tile([C, 135], dt.bfloat16, tag="wconv")
        nc.sync.dma_start(wconv_t[:], wconv_ext[:])
        bconv_t = cp.tile([15, 1], dt.float32, tag="bconv")
        nc.sync.dma_start(bconv_t[:], bconv_ext[:])
        dwt_t = cp.tile([128, 192], dt.bfloat16, tag="dwt")
        nc.sync.dma_start(dwt_t[:], dwt_ext[:])
        reg0_t = cp.tile([128, 9], dt.float32, tag="reg0")
        nc.sync.dma_start(reg0_t[:], reg0_ext[:])
        reg1_t = cp.tile([128, 9], dt.float32, tag="reg1")
        nc.sync.dma_start(reg1_t[:], reg1_ext[:])
        yc_t = cp.tile([128, NB], dt.float32, tag="yc")
        nc.sync.dma_start(yc_t[:], yc_ext[:])
        xc_t = cp.tile([128, NB], dt.float32, tag="xc")
        nc.sync.dma_start(xc_t[:], xc_ext[:])

        bn_in = {}
        for nm, ext in (("gamma", gamma_ext), ("beta", beta_ext),
                        ("rmean", rmean_ext), ("rvar", rvar_ext)):
            t = cp.tile([C, 1], dt.float32, tag=nm)
            nc.sync.dma_start(t[:], ext[:])
            bn_in[nm] = t

        id48 = cp.tile([C, C], dt.float32, tag="id48")
        make_identity(nc, id48[:])
        id16 = id48[0:15, 0:15]
        id128 = cp.tile([128, 128], dt.bfloat16, tag="id128")
        make_identity(nc, id128[:])

        # Dummy 128-descriptor gather issued first: forces the Pool engine's
        # gather-ucode LOAD_LIB (which barriers on all previously emitted
        # work) to happen at ~8us against an empty pipeline instead of right
        # before the first real gather.
        dz_idx = tp.tile([128, 8], dt.int16, tag="dz_idx")
        nc.gpsimd.memset(dz_idx[:], 0)
        dz_out = tp.tile([128, 256], dt.bfloat16, tag="dz_out")
        nc.gpsimd.dma_gather(
            out_ap=AP(dz_out.tensor, dz_out[:].offset,
                      [dz_out[:].ap[0], [QW, 1], [1, QW]]),
            in_ap=x_quad[:], idxs_ap=dz_idx[:],
            num_idxs=128, num_idxs_reg=128,
            elem_size=QW, single_packet=False)

        # ---------------- padded CHW copy (bf16) for convs ----------------
        x_pad = cp.tile([C, PADLEN], dt.bfloat16, tag="x_pad")
        nc.vector.memset(x_pad[:], 0.0)
        xpad_int = AP(x_pad.tensor, x_pad[:].offset + PADW + 1,
                      [x_pad[:].ap[0], [PADW, H], [1, W]])
        nc.vector.tensor_copy(xpad_int, x_sb[:])   # f32 -> bf16 cast on DVE

        # bn scale' = gamma * rsqrt(rvar+eps); shift' = beta - rmean*scale'
        veps = tp.tile([C, 1], dt.float32, tag="veps")
        nc.vector.tensor_scalar(veps[:], bn_in["rvar"][:], BN_EPS, None, op.add)
        vsq = tp.tile([C, 1], dt.float32, tag="vsq")
        nc.scalar.activation(vsq[:], veps[:], act.Sqrt)
        vri = tp.tile([C, 1], dt.float32, tag="vri")
        nc.vector.reciprocal(vri[:], vsq[:])
        scale_t = cp.tile([C, 1], dt.float32, tag="scale")
        nc.vector.tensor_tensor(scale_t[:], bn_in["gamma"][:], vri[:], op.mult)
        vms = tp.tile([C, 1], dt.float32, tag="vms")
        nc.vector.tensor_tensor(vms[:], bn_in["rmean"][:], scale_t[:], op.mult)
        shift_t = cp.tile([C, 1], dt.float32, tag="shift")
        nc.vector.tensor_tensor(shift_t[:], bn_in["beta"][:], vms[:], op.subtract)

        # ---------------- x -> HWC (bf16) via PE transposes ----------------
        x_hwc = cp.tile([128, NB * C], dt.bfloat16, tag="x_hwc")
        for g in range((NB + 3) // 4):          # 4 blocks per PSUM tile
            nblk = min(4, NB - g * 4)
            ps = pp_a.tile([128, 4 * C], dt.float32, name="psA", tag="psA")
            for j in range(nblk):
                b = g * 4 + j
                nc.tensor.transpose(ps[:, j * C:(j + 1) * C],
                                    x_sb[:, b * 128:(b + 1) * 128], id48[:])
            nc.scalar.activation(x_hwc[:, g * 4 * C:(g * 4 + nblk) * C],
                                 ps[:, :nblk * C], act.Copy)

        # ---------------- quad table to DRAM (column-major) ----------------
        # Table row r (= corner-quad of flat pixel p at slot shift) is stored
        # at DRAM elem offset (r%128)*(55*QW) + (r//128)*QW, i.e. partition-
        # major. The interleaved 512B rows are assembled in SBUF (DVE strided
        # copies from partition-rotated x_hwc copies) and written out as one
        # contiguous run per partition per slab -- ~512 large descriptors
        # instead of 27648 x 96B ones. The gather index becomes
        # idx' = 55*r - 7039*(r//128) so idx'*512B lands on the same cell.
        NCH = (RQ + 127) // 128          # 55 chunks per partition
        starts = {0: LEAD, 1: LEAD - 1, 2: LEAD - W, 3: LEAD - W - 1}
        pstride = x_hwc[:].ap[0][0]
        slabs = [(k, min(7, NCH - k)) for k in range(0, NCH, 7)]
        slab_out = []                    # per-slab out-DMA instruction

        def emit_slab(sidx):
            (k0, nk) = slabs[sidx]
            qsl = wp.tile([128, 7 * QW], dt.bfloat16, tag="qsl", bufs=2)
            nc.vector.memset(qsl[:], 0.0)
            for j in range(4):
                s = starts[j]
                xr = wp.tile([128, 7 * C], dt.bfloat16, tag="xr", bufs=2)
                # rect A: dst partitions [s,128), chunks [k0, min(k0+nk,54))
                ka0, ka1 = k0, min(k0 + nk, NCH - 1)
                if ka0 < ka1:
                    dst = AP(xr.tensor, xr[:].offset + s * xr[:].ap[0][0],
                             [[xr[:].ap[0][0], 128 - s], [1, (ka1 - ka0) * C]])
                    srcA = AP(x_hwc.tensor, x_hwc[:].offset + ka0 * C,
                              [[pstride, 128 - s], [1, (ka1 - ka0) * C]])
                    nc.sync.dma_start(out=dst, in_=srcA)
                # rect B: dst partitions [0,s), chunks [max(k0,1), k0+nk)
                kb0, kb1 = max(k0, 1), k0 + nk
                if kb0 < kb1 and s > 0:
                    dst = AP(xr.tensor, xr[:].offset + (kb0 - k0) * C,
                             [[xr[:].ap[0][0], s], [1, (kb1 - kb0) * C]])
                    srcB = AP(x_hwc.tensor,
                              x_hwc[:].offset + (128 - s) * pstride
                              + (kb0 - 1) * C,
                              [[pstride, s], [1, (kb1 - kb0) * C]])
                    nc.sync.dma_start(out=dst, in_=srcB)
                # interleave slot j into the slab rows
                nc.vector.tensor_copy(
                    AP(qsl.tensor, qsl[:].offset + j * 64,
                       [qsl[:].ap[0], [QW, nk], [1, C]]),
                    AP(xr.tensor, xr[:].offset,
                       [xr[:].ap[0], [C, nk], [1, C]]))
            wo = nc.sync.dma_start(
                out=AP(x_quad, k0 * QW, [[NCH * QW, 128], [1, nk * QW]]),
                in_=qsl[:, :nk * QW])
            slab_out.append(wo)

        emit_slab(0)
        emit_slab(1)

        def _slice_slabs(b0, nb):
            p_lo, p_hi = b0 * 128, (b0 + nb) * 128
            y_lo, y_hi = p_lo // W, (p_hi - 1) // W
            r_lo = max(0, LEAD + (y_lo - MAXD) * W)
            r_hi = min(RQ - 1, LEAD + (y_hi + MAXD + 1) * W + W - 1)
            out = []
            for sidx, (k0, nk) in enumerate(slabs):
                if k0 * 128 <= r_hi and r_lo < (k0 + nk) * 128:
                    out.append(sidx)
            return out

        # ---------------- convs: 15ch 3x3 via 9 accumulated matmuls --------
        conv_sb = cp.tile([15, HW], dt.float32, tag="conv_sb")
        trows = [(t * CONV_ROWS, min(CONV_ROWS, H - t * CONV_ROWS))
                 for t in range((H + CONV_ROWS - 1) // CONV_ROWS)]

        def conv_tile(r0, nr):
            psc = pp_a.tile([15, CONV_ROWS * W], dt.float32, name="psA2", tag="psA2",
                            bufs=3)
            npx = nr * W
            for s in range(9):
                r, c = divmod(s, 3)
                rhs = AP(x_pad.tensor,
                         x_pad[:].offset + (r0 + r) * PADW + c,
                         [x_pad[:].ap[0], [PADW, nr], [1, W]])
                nc.tensor.matmul(psc[:, :npx], wconv_t[:, s * 15:(s + 1) * 15],
                                 rhs, start=(s == 0), stop=(s == 8))
            nc.scalar.activation(conv_sb[:, r0 * W:r0 * W + npx], psc[:, :npx],
                                 act.Identity, bias=bconv_t[:])

        # conv output -> pixel-major (tcols), per 8-block group
        tcols = cp.tile([128, NB * 15], dt.float32, tag="tcols")

        def tcols_group(g):
            nblk = min(8, NB - g * 8)
            ps = pp_a.tile([128, 8 * 15], dt.float32, name="psA", tag="psA")
            for j in range(nblk):
                b = g * 8 + j
                nc.tensor.transpose(ps[:, j * 15:(j + 1) * 15],
                                    conv_sb[:, b * 128:(b + 1) * 128],
                                    id16[:])
            nc.scalar.activation(tcols[:, g * 8 * 15:(g * 8 + nblk) * 15],
                                 ps[:, :nblk * 15], act.Copy)

        # ---------------- bulk offset / weight / index math ----------------
        def plane(tag):
            return tp.tile([128, QTOT], dt.float32, name=tag, tag=tag)

        py = plane("py"); px = plane("px")
        t1 = plane("t1"); t2 = plane("t2"); t3 = plane("t3")
        fy = plane("fy"); fx = plane("fx")
        y0 = plane("y0"); x0 = plane("x0")
        wy0 = py; wy1 = px                       # reuse dead slots (disjoint cols)
        wx0 = plane("wx0"); wx1 = plane("wx1")
        idxf = fy                                # reuse dead slot
        cvt_i = tp.tile([128, QTOT], dt.int32, name="cvt_i", tag="cvt_i")
        w4 = cp.tile([128, QTOT * 4], dt.bfloat16, tag="w4")
        idx_t = cp.tile([128, QTOT], dt.int16, tag="idx")
        fold = cp.tile([16, 8 * QTOT], dt.int16, tag="fold")
        # one idxw tile per offset-math group, so a slice gather depends only
        # on its own group's fold (Tile tracks whole-tile versions)
        idxw_tiles = {}
        groups = [(0, 4), (4, 4), (8, 8), (16, 16), (32, 22)]

        def emit_group(g0, gn):
            nq = gn * 9
            c0 = g0 * 9
            sl = slice(c0, c0 + nq)

            def tcol_b(ch):
                return AP(tcols.tensor, tcols[:].offset + ch + g0 * 15,
                          [tcols[:].ap[0], [15, gn], [0, 9]])

            def reg_b(t):
                return AP(t.tensor, t[:].offset, [t[:].ap[0], [0, gn], [1, 9]])

            def coord_b(t):
                return AP(t.tensor, t[:].offset + g0,
                          [t[:].ap[0], [1, gn], [0, 9]])

            tt = nc.vector.tensor_tensor
            ts = nc.vector.tensor_scalar
            tt(t1[:, sl], reg_b(reg0_t), tcol_b(0), op.mult)
            tt(t2[:, sl], reg_b(reg1_t), tcol_b(1), op.mult)
            tt(t3[:, sl], t1[:, sl], t2[:, sl], op.add)
            tt(t1[:, sl], t3[:, sl], tcol_b(4), op.add)
            tt(py[:, sl], t1[:, sl], coord_b(yc_t), op.add)
            tt(t1[:, sl], reg_b(reg0_t), tcol_b(2), op.mult)
            tt(t2[:, sl], reg_b(reg1_t), tcol_b(3), op.mult)
            tt(t3[:, sl], t1[:, sl], t2[:, sl], op.add)
            tt(t1[:, sl], t3[:, sl], tcol_b(5), op.add)
            tt(px[:, sl], t1[:, sl], coord_b(xc_t), op.add)

            # floor(v) = int(v) - (v < int(v)); rounding-mode agnostic
            for (v, fl, fr) in ((py, y0, fy), (px, x0, fx)):
                nc.vector.tensor_copy(cvt_i[:, sl], v[:, sl])
                nc.vector.tensor_copy(t1[:, sl], cvt_i[:, sl])
                tt(t2[:, sl], v[:, sl], t1[:, sl], op.is_lt)
                tt(fl[:, sl], t1[:, sl], t2[:, sl], op.subtract)
                tt(fr[:, sl], v[:, sl], fl[:, sl], op.subtract)

            mk_b = AP(tcols.tensor, tcols[:].offset + 6 + g0 * 15,
                      [tcols[:].ap[0], [15, gn], [1, 9]])
            ts(t1[:, sl], y0[:, sl], 0.0, None, op.is_ge)
            ts(t2[:, sl], y0[:, sl], float(H - 1), None, op.is_le)
            tt(t3[:, sl], t1[:, sl], t2[:, sl], op.mult)
            ts(t1[:, sl], fy[:, sl], -1.0, 1.0, op.mult, op.add)
            tt(wy0[:, sl], t1[:, sl], t3[:, sl], op.mult)
            ts(t1[:, sl], y0[:, sl], -1.0, None, op.is_ge)
            ts(t2[:, sl], y0[:, sl], float(H - 2), None, op.is_le)
            tt(t3[:, sl], t1[:, sl], t2[:, sl], op.mult)
            tt(wy1[:, sl], fy[:, sl], t3[:, sl], op.mult)
            ts(t1[:, sl], x0[:, sl], 0.0, None, op.is_ge)
            ts(t2[:, sl], x0[:, sl], float(W - 1), None, op.is_le)
            tt(t3[:, sl], t1[:, sl], t2[:, sl], op.mult)
            ts(t1[:, sl], fx[:, sl], -1.0, 1.0, op.mult, op.add)
            tt(wx0[:, sl], t1[:, sl], t3[:, sl], op.mult)
            ts(t1[:, sl], x0[:, sl], -1.0, None, op.is_ge)
            ts(t2[:, sl], x0[:, sl], float(W - 2), None, op.is_le)
            tt(t3[:, sl], t1[:, sl], t2[:, sl], op.mult)
            tt(wx1[:, sl], fx[:, sl], t3[:, sl], op.mult)
            tt(wy0[:, sl], wy0[:, sl], mk_b, op.mult)
            tt(wy1[:, sl], wy1[:, sl], mk_b, op.mult)

            def w4_slot(j):
                return AP(w4.tensor, w4[:].offset + j + c0 * 4,
                          [w4[:].ap[0], [4, nq]])
            tt(w4_slot(0), wy0[:, sl], wx0[:, sl], op.mult)
            tt(w4_slot(1), wy0[:, sl], wx1[:, sl], op.mult)
            tt(w4_slot(2), wy1[:, sl], wx0[:, sl], op.mult)
            tt(w4_slot(3), wy1[:, sl], wx1[:, sl], op.mult)

            # gather index: r = clip(LEAD + y0*W + x0, 0, HW+W+1), then the
            # column-major remap idx' = 55*r - 7039*floor(r/128), int16
            nc.vector.scalar_tensor_tensor(idxf[:, sl], y0[:, sl], float(W),
                                           x0[:, sl], op.mult, op.add)
            ts(idxf[:, sl], idxf[:, sl], float(LEAD), 0.0, op.add, op.max)
            ts(idxf[:, sl], idxf[:, sl], float(HW + W + 1), None, op.min)
            ts(t1[:, sl], idxf[:, sl], 1.0 / 128.0, None, op.mult)
            nc.vector.tensor_copy(cvt_i[:, sl], t1[:, sl])
            nc.vector.tensor_copy(t2[:, sl], cvt_i[:, sl])
            tt(t3[:, sl], t1[:, sl], t2[:, sl], op.is_lt)
            tt(t2[:, sl], t2[:, sl], t3[:, sl], op.subtract)
            ts(t2[:, sl], t2[:, sl], float(NCH * 128 - 1), None, op.mult)
            nc.vector.scalar_tensor_tensor(idxf[:, sl], idxf[:, sl],
                                           float(NCH), t2[:, sl],
                                           op.mult, op.subtract)
            nc.vector.tensor_copy(idx_t[:, sl], idxf[:, sl])

            # 16-wrap via DRAM round-trip fold + in-partition interleave
            wr = nc.sync.dma_start(
                out=AP(idx_dram, c0, [[QTOT, 128], [1, nq]]),
                in_=idx_t[:, sl])
            rdf = nc.sync.dma_start(
                out=AP(fold.tensor, fold[:].offset + c0,
                       [fold[:].ap[0], [QTOT, 8], [1, nq]]),
                in_=AP(idx_dram, c0, [[QTOT, 16], [16 * QTOT, 8], [1, nq]]))
            add_dep_helper(rdf.ins, wr.ins, sync=False, reason="idx fold rt")
            idxw = cp.tile([128, nq * 8], dt.int16, tag=f"idxw{g0}")
            idxw_tiles[g0] = (g0, gn, idxw)
            i16 = idxw[0:16, :]
            nc.vector.tensor_copy(
                AP(i16.tensor, i16.offset,
                   [i16.ap[0], [1, 8], [8, nq]]),
                AP(fold.tensor, fold[:].offset + c0,
                   [fold[:].ap[0], [QTOT, 8], [1, nq]]))
            for g in range(1, 8):
                nc.sync.dma_start(
                    out=idxw[16 * g:16 * (g + 1), :],
                    in_=idxw[0:16, :])

        # Interleave conv tiles / tcols groups / offset-math groups so the
        # first gather can issue early. Group plan (blocks): 6, 6, 6, 18, 18.
        # tcols 8-block group t reads conv pixels up to min(8t+8, NB)*128,
        # so emit exactly the conv tiles covering those rows first.
        n_tc_groups = (NB + 7) // 8
        conv_done = 0        # conv tiles emitted
        tcols_done = 0       # tcols 8-block groups emitted
        for gi_, (g0, gn) in enumerate(groups):
            need_tc = min((g0 + gn + 7) // 8, n_tc_groups)
            while tcols_done < need_tc:
                last_px = min(8 * tcols_done + 8, NB) * 128 - 1
                need_tile = min((last_px // W) // CONV_ROWS + 1, len(trows))
                while conv_done < need_tile:
                    conv_tile(*trows[conv_done])
                    conv_done += 1
                tcols_group(tcols_done)
                tcols_done += 1
            emit_group(g0, gn)
            if gi_ == 1:
                for s_ in range(2, len(slabs)):
                    emit_slab(s_)
        while conv_done < len(trows):
            conv_tile(*trows[conv_done])
            conv_done += 1

        # ---------------- gather / combine / matmul / epilogue -------------
        _slab_waited = set()
        for si, (b0, nb) in enumerate(_slices()):
            Q = nb * 9
            npx = nb * 128
            q0 = b0 * 9
            gq = wp.tile([128, SLICE_BLOCKS * 9 * QW], dt.bfloat16, tag="gq",
                         bufs=3)
            gq3 = AP(gq.tensor, gq[:].offset,
                     [gq[:].ap[0], [QW, Q], [1, QW]])
            grp = next(v for v in idxw_tiles.values()
                       if v[0] <= b0 and b0 + nb <= v[0] + v[1])
            lq = (b0 - grp[0]) * 9
            gi = nc.gpsimd.dma_gather(
                out_ap=gq3,
                in_ap=x_quad[:],
                idxs_ap=grp[2][:, lq * 8:(lq + Q) * 8],
                num_idxs=Q * 128,
                num_idxs_reg=Q * 128,
                elem_size=QW,
                single_packet=False,
            )
            for sidx in _slice_slabs(b0, nb):
                add_dep_helper(gi.ins, slab_out[sidx].ins,
                               reason="gather after quad slab")

            # weighted 4-corner combine (in-place over the gathered tile;
            # slot pads multiply garbage that is never read)
            prod = gq
            gview = gq[:, :Q * QW]
            wb = AP(w4.tensor, w4[:].offset + q0 * 4,
                    [w4[:].ap[0], [1, Q * 4], [0, 64]])
            nc.vector.tensor_tensor(gview, gview, wb, op.mult)

            def pview(off):
                return AP(prod.tensor, prod[:].offset + off,
                          [prod[:].ap[0], [QW, Q], [1, C]])
            sa = wp.tile([128, SLICE_BLOCKS * 9 * C], dt.bfloat16, tag="sa")
            # sampt doubles as scratch for the second corner-pair sum; its
            # cols are rewritten only after the final add consumed them
            sampt = wp.tile([128, 4 * SLICE_BLOCKS * 128], dt.bfloat16, tag="sampt")
            sb2 = sampt
            nc.vector.tensor_tensor(sa[:, :Q * C], pview(0), pview(64), op.add)
            nc.vector.tensor_tensor(sb2[:, :Q * C], pview(128), pview(192),
                                    op.add)
            samp = sa
            nc.vector.tensor_tensor(samp[:, :Q * C], sa[:, :Q * C],
                                    sb2[:, :Q * C], op.add)
            for ib in range(nb):
                ps = pp_st.tile([128, 512], dt.bfloat16, tag="ps_st")
                base = ib * 9 * C
                for ch in range(3):
                    nc.tensor.transpose(
                        ps[:, ch * 128:(ch + 1) * 128],
                        samp[:, base + ch * 128: base + (ch + 1) * 128],
                        id128[:])
                nc.tensor.transpose(ps[0:C, 384:512],
                                    samp[:, base + 384: base + 432], id128[:])
                dst02 = AP(sampt.tensor, sampt[:].offset + ib * 128,
                           [sampt[:].ap[0], [npx, 3], [1, 128]])
                nc.scalar.activation(dst02, ps[:, 0:384], act.Copy)
                nc.scalar.activation(sampt[0:C, 3 * npx + ib * 128:
                                           3 * npx + (ib + 1) * 128],
                                     ps[0:C, 384:512], act.Copy)

            # matmul: out[o, px] += dwT_chunk.T @ sampt_chunk
            pso = pp_out.tile([C, SLICE_BLOCKS * 128], dt.float32, tag="ps_out")
            for ch in range(3):
                nc.tensor.matmul(pso[:, :npx], dwt_t[:, ch * C:(ch + 1) * C],
                                 sampt[:, ch * npx:(ch + 1) * npx],
                                 start=(ch == 0), stop=False)
            nc.tensor.matmul(pso[:, :npx], dwt_t[0:C, 3 * C:4 * C],
                             sampt[0:C, 3 * npx:3 * npx + npx],
                             start=False, stop=True)

            # epilogue: BN(running stats) + residual + relu -> DRAM
            bno = wp.tile([C, SLICE_BLOCKS * 128], dt.float32, tag="bno")
            nc.scalar.activation(bno[:, :npx], pso[:, :npx], act.Identity,
                                 bias=shift_t[:], scale=scale_t[:])
            nc.vector.tensor_tensor(bno[:, :npx], bno[:, :npx],
                                    x_sb[:, b0 * 128:b0 * 128 + npx], op.add)
            nc.vector.tensor_scalar(bno[:, :npx], bno[:, :npx], 0.0, None,
                                    op.max)
            nc.sync.dma_start(out_ext[:, b0 * 128:b0 * 128 + npx],
                              bno[:, :npx])

    nc.compile()
    return nc


def _host_pack(inputs):
    """Weight/constant layout prep (no input-data compute)."""
    bf16 = ml_dtypes.bfloat16
    wa = np.concatenate([inputs['tm_w'], inputs['tr_w'], inputs['mk_w']],
                        axis=0)                       # [15, C, 3, 3]
    wconv = np.ascontiguousarray(
        wa.transpose(1, 2, 3, 0).reshape(C, 9 * 15)).astype(bf16)
    bconv = np.concatenate([inputs['tm_b'], inputs['tr_b'],
                            inputs['mk_b']]).astype(np.float32)
    dwr = inputs['dw'].reshape(C, C, 9)               # [o, c, k]
    dwT = np.zeros((512, C), dtype=np.float32)
    dwT[:432] = dwr.transpose(2, 1, 0).reshape(9 * C, C)
    dwt = np.ascontiguousarray(
        dwT.reshape(4, 128, C).transpose(1, 0, 2).reshape(128, 4 * C)
    ).astype(bf16)
    reg0 = np.tile(_REG[0], (128, 1)).astype(np.float32)
    reg1 = np.tile(_REG[1], (128, 1)).astype(np.float32)
    pix = (np.arange(NB)[None, :] * 128 + np.arange(128)[:, None])
    ycoord = (pix // W).astype(np.float32)
    xcoord = (pix % W).astype(np.float32)
    shared = dict(
        wconv=wconv, bconv=bconv, dwt=dwt, reg0=reg0, reg1=reg1,
        ycoord=ycoord, xcoord=xcoord,
        gamma=inputs['gamma'].astype(np.float32),
        beta=inputs['beta'].astype(np.float32),
        rmean=inputs['rmean'].astype(np.float32),
        rvar=inputs['rvar'].astype(np.float32),
    )
    return shared


def kernel(**inputs):
    inputs = {k: np.asarray(v) for k, v in inputs.items()}
    if 'nc' not in _built:
        _built['nc'] = build_nc()
    nc = _built['nc']

    from concourse.bass_utils import run_bass_kernel_spmd
    shared = _host_pack(inputs)
    x = inputs['x'].astype(np.float32)
    in_maps = []
    for i in range(N):
        m = dict(shared)
        m['x'] = np.ascontiguousarray(x[i].reshape(C, HW))
        in_maps.append(m)
    res = run_bass_kernel_spmd(nc, in_maps, core_ids=list(range(N)))
    out = np.stack([res.results[i]['out'].reshape(C, H, W)
                    for i in range(N)])
    return out.astype(np.float32)



# revision 3
# speedup vs baseline: 1.0191x; 1.0191x over previous
"""Trainium2 Bass kernel for nn_AdaptBlockV2 (deformable-conv-v2 block).

Data-parallel over the batch axis: 8 samples -> 8 NeuronCores, one sample
per core. Inside each core:
  A) load x; build zero-padded CHW copy (bf16) for the convs; transpose x to
     HWC (bf16, three band tiles) and write a ROW-MAJOR "quad" gather table
     straight to DRAM with 4 strided DMAs per band (row r = channels of flat
     pixels [r, r+1, r+W, r+W+1] at slots 0..3) -- one indirect-DMA
     descriptor then fetches all 4 bilinear corners of one (pixel, tap).
     Table edge rows are pre-zeroed so zero-weight fetches read 0, not NaN.
  B) 15-channel 3x3 conv (offset transform T, translation tr, modulation
     mask) as 9 PSUM-accumulated matmuls; transpose conv output to
     pixel-major; bulk DVE math for sampling positions py/px, floor via
     floored-mod, corner weights (bilinear x mask x validity), and the flat
     gather index (= table row, no remap needed with the row-major table).
  C) per-slice pipeline: indirect DMA gather -> DVE weighted 4-corner
     combine -> PE transpose of samp to (tap,channel)-major -> matmul with
     dw -> BN (running stats) + residual (DVE add) + ReLU (Scalar engine)
     -> DMA out.

The SWDGE gather descriptor generation on the Pool engine (~8ns/descriptor
x 62208 descriptors ~= 490us) is the hard floor; the prologue is kept to
~25us by minimizing serial Sync-engine DMA issues (the quad table is 10
DMAs instead of ~72; the idx 16-wrap fold is 3 DMAs per group instead of 9
-- queue 0 only needs the indices replicated to partitions 0..31).

kernel(**inputs) takes FULL unsharded inputs, returns the FULL output.
"""
import numpy as np
import ml_dtypes

N, C, H, W = 8, 48, 96, 72
HW = H * W                       # 6912
LEAD = W + 2                     # 74: lead pad rows in the quad table
RQ = 7040                        # quad-table rows (>= HW + W + 2)
QW = 256                         # quad-table row width (512B, dma_gather)
NB = HW // 128                   # 54 pixel blocks
QTOT = NB * 9                    # 486 (block, tap) chunks
PADW = W + 2                     # 74 padded conv row stride
PADLEN = (H + 2) * PADW         # 7252
BN_EPS = 1e-5
CONV_ROWS = 7                    # conv N-tile = 7 image rows = 504 pixels
SLICE_BLOCKS = 4                 # gather/combine slice = 4 pixel blocks
MAXD = 10                        # sample-displacement bound for band deps
BANDS = [(0, 12), (12, 32), (32, 54)]   # x_hwc chunk ranges per table band
DELTA = [0, 1, W, W + 1]                # quad slot pixel offsets

_REG = np.array([[-1, -1, -1, 0, 0, 0, 1, 1, 1],
                 [-1, 0, 1, -1, 0, 1, -1, 0, 1]], dtype=np.float32)

_built = {}


def _slices():
    out = []
    b = 0
    while b < NB:
        nb = min(SLICE_BLOCKS, NB - b)
        out.append((b, nb))
        b += nb
    return out


def build_nc():
    import concourse.bass as bass
    import concourse.bacc as bacc
    import concourse.tile as tile
    from concourse import mybir
    from concourse.bass import AP
    from concourse.masks import make_identity
    from concourse.tile import add_dep_helper
    from contextlib import ExitStack

    dt = mybir.dt
    op = mybir.AluOpType
    act = mybir.ActivationFunctionType

    nc = bacc.Bacc("TRN2", target_bir_lowering=False, debug=False,
                   num_devices=N, dynamic_dma_scratch_size=16384)
    x_ext = nc.declare_dram_parameter("x", [C, HW], dt.float32, isOutput=False)
    blob16_ext = nc.declare_dram_parameter("blob16", [128, 327], dt.bfloat16,
                                           isOutput=False)
    blob32_ext = nc.declare_dram_parameter("blob32", [128, 131], dt.float32,
                                           isOutput=False)
    out_ext = nc.declare_dram_parameter("out", [C, HW], dt.float32, isOutput=True)

    x_quad = nc.dram_tensor("x_quad", [RQ, QW], dt.bfloat16)
    idx_dram = nc.dram_tensor("idx_dram", [128 * QTOT], dt.int16)

    with tile.TileContext(nc) as tc, ExitStack() as ctx:
        cp = ctx.enter_context(tc.tile_pool(name="const", bufs=1))
        tp = ctx.enter_context(tc.tile_pool(name="tmp", bufs=1))
        wp = ctx.enter_context(tc.tile_pool(name="work", bufs=2))
        pp_a = ctx.enter_context(tc.tile_pool(name="ps_a", bufs=2, space="PSUM"))
        pp_st = ctx.enter_context(tc.tile_pool(name="ps_st", bufs=2, space="PSUM"))
        pp_out = ctx.enter_context(tc.tile_pool(name="ps_out", bufs=1, space="PSUM"))

        # ---------------- constants / weights to SBUF ----------------
        x_sb = cp.tile([C, HW], dt.float32, tag="x_sb")
        nc.sync.dma_start(x_sb[:], x_ext[:])
        blob16 = cp.tile([128, 327], dt.bfloat16, tag="blob16")
        nc.sync.dma_start(blob16[:], blob16_ext[:])
        blob32 = cp.tile([128, 131], dt.float32, tag="blob32")
        nc.sync.dma_start(blob32[:], blob32_ext[:])

        b16p = blob16[:].ap[0][0]
        b16o = blob16[:].offset
        b32p = blob32[:].ap[0][0]
        b32o = blob32[:].offset

        def wconv_ap(s):         # [48, 15] stationary for conv tap s
            return AP(blob16.tensor, b16o + s * 15, [[b16p, C], [1, 15]])

        def dwt_ap(ch, nparts):  # [nparts, 48] stationary for dw chunk ch
            return AP(blob16.tensor, b16o + 135 + ch * C, [[b16p, nparts], [1, C]])

        def b32col(col, nparts):
            return AP(blob32.tensor, b32o + col, [[b32p, nparts], [1, 1]])

        bconv_ap = b32col(126, 15)

        id48 = cp.tile([C, C], dt.float32, tag="id48")
        make_identity(nc, id48[:])
        id16 = id48[0:15, 0:15]
        id128 = cp.tile([128, 128], dt.bfloat16, tag="id128")
        make_identity(nc, id128[:])

        # Dummy 128-descriptor gather issued first: forces the Pool engine's
        # gather-ucode LOAD_LIB (which barriers on all previously emitted
        # work) to happen immediately against an empty pipeline instead of
        # right before the first real gather.
        dz_idx = tp.tile([128, 8], dt.int16, tag="dz_idx")
        nc.gpsimd.memset(dz_idx[:], 0)
        dz_out = tp.tile([128, 256], dt.bfloat16, tag="dz_out")
        nc.gpsimd.dma_gather(
            out_ap=AP(dz_out.tensor, dz_out[:].offset,
                      [dz_out[:].ap[0], [QW, 1], [1, QW]]),
            in_ap=x_quad[:], idxs_ap=dz_idx[:],
            num_idxs=128, num_idxs_reg=128,
            elem_size=QW, single_packet=False)

        # ---------------- zero the quad-table edge rows ----------------
        # Rows [0, LEAD) and [RQ-127, RQ) can be fetched (clamped idx /
        # out-of-image corners) with zero weight; they must hold 0, not junk.
        ztile = tp.tile([128, 254], dt.bfloat16, tag="ztile")
        nc.vector.memset(ztile[:], 0.0)
        zlow = nc.sync.dma_start(
            out=AP(x_quad, 0, [[148, 128], [1, 148]]),
            in_=ztile[:, :148])
        zhigh = nc.sync.dma_start(
            out=AP(x_quad, (RQ - 127) * QW, [[254, 128], [1, 254]]),
            in_=ztile[:, :254])

        # ---------------- padded CHW copy (bf16) for convs ----------------
        # Only the pad cells are memset (head row+corner, right|left pad
        # pairs, tail), then the interior is cast-copied from x_sb.
        x_pad = cp.tile([C, PADLEN], dt.bfloat16, tag="x_pad")
        nc.vector.memset(x_pad[:, 0:PADW + 1], 0.0)
        nc.vector.memset(
            AP(x_pad.tensor, x_pad[:].offset + 2 * PADW - 1,
               [x_pad[:].ap[0], [PADW, H - 1], [1, 2]]), 0.0)
        nc.vector.memset(x_pad[:, (H + 1) * PADW - 1:PADLEN], 0.0)
        xpad_int = AP(x_pad.tensor, x_pad[:].offset + PADW + 1,
                      [x_pad[:].ap[0], [PADW, H], [1, W]])
        nc.vector.tensor_copy(xpad_int, x_sb[:])   # f32 -> bf16 cast on DVE

        # ---------------- x -> HWC (bf16) via PE transposes ----------------
        # One tile per table band so band writes only dep on their own chunks.
        hwc_tiles = [cp.tile([128, (k1 - k0) * C], dt.bfloat16,
                             name=f"x_hwc{i}", tag=f"x_hwc{i}")
                     for i, (k0, k1) in enumerate(BANDS)]

        def hwc_of(chunk):
            for (k0, k1), t in zip(BANDS, hwc_tiles):
                if k0 <= chunk < k1:
                    return t, k0
            raise AssertionError

        def emit_trans(g):                    # 4 blocks per PSUM tile
            nblk = min(4, NB - g * 4)
            ps = pp_a.tile([128, 4 * C], dt.float32, name="psA", tag="psA")
            for j in range(nblk):
                b = g * 4 + j
                nc.tensor.transpose(ps[:, j * C:(j + 1) * C],
                                    x_sb[:, b * 128:(b + 1) * 128], id48[:])
            t, k0 = hwc_of(g * 4)
            nc.scalar.activation(
                t[:, (g * 4 - k0) * C:(g * 4 - k0 + nblk) * C],
                ps[:, :nblk * C], act.Copy)

        # ---------------- quad table: 4 slot DMAs per band -----------------
        band_ins = {}

        def emit_band(bi):
            k0, k1 = BANDS[bi]
            nk = k1 - k0
            t, tk0 = hwc_of(k0)
            ws = []
            for j, dj in enumerate(DELTA):
                base = (LEAD - dj) * QW + 64 * j
                w = nc.sync.dma_start(
                    out=AP(x_quad, base + k0 * 128 * QW,
                           [[QW, 128], [128 * QW, nk], [1, C]]),
                    in_=AP(t.tensor, t[:].offset + (k0 - tk0) * C,
                           [t[:].ap[0], [C, nk], [1, C]]))
                if bi == 0:
                    add_dep_helper(w.ins, zlow.ins, reason="slot after zero")
                if bi == 2:
                    add_dep_helper(w.ins, zhigh.ins, reason="slot after zero")
                ws.append(w)
            band_ins[bi] = ws

        def _slice_table_deps(b0, nb):
            y_lo = (b0 * 128) // W
            y_hi = ((b0 + nb) * 128 - 1) // W
            r_lo = max(0, LEAD + (y_lo - MAXD) * W)
            r_hi = min(RQ - 1, LEAD + (y_hi + MAXD + 1) * W + W - 1)
            deps = []
            if r_lo < LEAD:
                deps.append(zlow)
            if r_hi >= RQ - 127:
                deps.append(zhigh)
            for bi, (k0, k1) in enumerate(BANDS):
                if r_lo <= k1 * 128 + W + 1 and r_hi >= k0 * 128 + 1:
                    deps.extend(band_ins[bi])
            return deps

        # bn scale' = gamma * rsqrt(rvar+eps); shift' = beta - rmean*scale'
        def emit_bn():
            veps = tp.tile([C, 1], dt.float32, tag="veps")
            nc.vector.tensor_scalar(veps[:], b32col(130, C), BN_EPS, None, op.add)
            vsq = tp.tile([C, 1], dt.float32, tag="vsq")
            nc.scalar.activation(vsq[:], veps[:], act.Sqrt)
            vri = tp.tile([C, 1], dt.float32, tag="vri")
            nc.vector.reciprocal(vri[:], vsq[:])
            scale_t = cp.tile([C, 1], dt.float32, tag="scale")
            nc.vector.tensor_tensor(scale_t[:], b32col(127, C), vri[:], op.mult)
            vms = tp.tile([C, 1], dt.float32, tag="vms")
            nc.vector.tensor_tensor(vms[:], b32col(129, C), scale_t[:], op.mult)
            shift_t = cp.tile([C, 1], dt.float32, tag="shift")
            nc.vector.tensor_tensor(shift_t[:], b32col(128, C), vms[:], op.subtract)
            return scale_t, shift_t

        # ---------------- convs: 15ch 3x3 via 9 accumulated matmuls --------
        conv_sb = cp.tile([15, HW], dt.float32, tag="conv_sb")
        trows = [(t * CONV_ROWS, min(CONV_ROWS, H - t * CONV_ROWS))
                 for t in range((H + CONV_ROWS - 1) // CONV_ROWS)]

        def conv_tile(r0, nr):
            psc = pp_a.tile([15, CONV_ROWS * W], dt.float32, name="psA2",
                            tag="psA2", bufs=3)
            npx = nr * W
            for s in range(9):
                r, c = divmod(s, 3)
                rhs = AP(x_pad.tensor,
                         x_pad[:].offset + (r0 + r) * PADW + c,
                         [x_pad[:].ap[0], [PADW, nr], [1, W]])
                nc.tensor.matmul(psc[:, :npx], wconv_ap(s),
                                 rhs, start=(s == 0), stop=(s == 8))
            nc.scalar.activation(conv_sb[:, r0 * W:r0 * W + npx], psc[:, :npx],
                                 act.Identity, bias=bconv_ap)

        # conv output -> pixel-major (tcols), per 8-block group
        tcols = cp.tile([128, NB * 15], dt.float32, tag="tcols")

        def tcols_group(g):
            nblk = min(8, NB - g * 8)
            ps = pp_a.tile([128, 8 * 15], dt.float32, name="psA", tag="psA")
            for j in range(nblk):
                b = g * 8 + j
                nc.tensor.transpose(ps[:, j * 15:(j + 1) * 15],
                                    conv_sb[:, b * 128:(b + 1) * 128],
                                    id16[:])
            nc.scalar.activation(tcols[:, g * 8 * 15:(g * 8 + nblk) * 15],
                                 ps[:, :nblk * 15], act.Copy)

        # ---------------- bulk offset / weight / index math ----------------
        def plane(tag):
            return tp.tile([128, QTOT], dt.float32, name=tag, tag=tag)

        py = plane("py"); px = plane("px")
        t1 = plane("t1"); t2 = plane("t2"); t3 = plane("t3")
        fy = plane("fy"); fx = plane("fx")
        y0 = plane("y0"); x0 = plane("x0")
        wy0 = py; wy1 = px                       # reuse dead slots (disjoint cols)
        wx0 = plane("wx0"); wx1 = plane("wx1")
        idxf = fy                                # reuse dead slot
        cvt_i = tp.tile([128, QTOT], dt.int32, name="cvt_i", tag="cvt_i")
        w4 = cp.tile([128, QTOT * 4], dt.bfloat16, tag="w4")
        idx_t = cp.tile([128, QTOT], dt.int16, tag="idx")
        fold = cp.tile([16, 8 * QTOT], dt.int16, tag="fold")
        # one idxw tile per offset-math group, so a slice gather depends only
        # on its own group's fold (Tile tracks whole-tile versions)
        idxw_tiles = {}
        groups = [(0, 4), (4, 4), (8, 8), (16, 16), (32, 22)]

        def emit_group(g0, gn):
            nq = gn * 9
            c0 = g0 * 9
            sl = slice(c0, c0 + nq)

            def tcol_b(ch):
                return AP(tcols.tensor, tcols[:].offset + ch + g0 * 15,
                          [tcols[:].ap[0], [15, gn], [0, 9]])

            def reg_b(roff):
                return AP(blob32.tensor, b32o + roff,
                          [[b32p, 128], [0, gn], [1, 9]])

            def coord_b(coff):
                return AP(blob32.tensor, b32o + coff + g0,
                          [[b32p, 128], [1, gn], [0, 9]])

            tt = nc.vector.tensor_tensor
            ts = nc.vector.tensor_scalar
            tt(t1[:, sl], reg_b(0), tcol_b(0), op.mult)
            tt(t2[:, sl], reg_b(9), tcol_b(1), op.mult)
            tt(t3[:, sl], t1[:, sl], t2[:, sl], op.add)
            tt(t1[:, sl], t3[:, sl], tcol_b(4), op.add)
            tt(py[:, sl], t1[:, sl], coord_b(18), op.add)
            tt(t1[:, sl], reg_b(0), tcol_b(2), op.mult)
            tt(t2[:, sl], reg_b(9), tcol_b(3), op.mult)
            tt(t3[:, sl], t1[:, sl], t2[:, sl], op.add)
            tt(t1[:, sl], t3[:, sl], tcol_b(5), op.add)
            tt(px[:, sl], t1[:, sl], coord_b(72), op.add)

            # floor(v) = int(v) - (v < int(v)); rounding-mode agnostic
            for (v, fl, fr) in ((py, y0, fy), (px, x0, fx)):
                nc.vector.tensor_copy(cvt_i[:, sl], v[:, sl])
                nc.vector.tensor_copy(t1[:, sl], cvt_i[:, sl])
                tt(t2[:, sl], v[:, sl], t1[:, sl], op.is_lt)
                tt(fl[:, sl], t1[:, sl], t2[:, sl], op.subtract)
                tt(fr[:, sl], v[:, sl], fl[:, sl], op.subtract)

            mk_b = AP(tcols.tensor, tcols[:].offset + 6 + g0 * 15,
                      [tcols[:].ap[0], [15, gn], [1, 9]])
            ts(t1[:, sl], y0[:, sl], 0.0, None, op.is_ge)
            ts(t2[:, sl], y0[:, sl], float(H - 1), None, op.is_le)
            tt(t3[:, sl], t1[:, sl], t2[:, sl], op.mult)
            ts(t1[:, sl], fy[:, sl], -1.0, 1.0, op.mult, op.add)
            tt(wy0[:, sl], t1[:, sl], t3[:, sl], op.mult)
            ts(t1[:, sl], y0[:, sl], -1.0, None, op.is_ge)
            ts(t2[:, sl], y0[:, sl], float(H - 2), None, op.is_le)
            tt(t3[:, sl], t1[:, sl], t2[:, sl], op.mult)
            tt(wy1[:, sl], fy[:, sl], t3[:, sl], op.mult)
            ts(t1[:, sl], x0[:, sl], 0.0, None, op.is_ge)
            ts(t2[:, sl], x0[:, sl], float(W - 1), None, op.is_le)
            tt(t3[:, sl], t1[:, sl], t2[:, sl], op.mult)
            ts(t1[:, sl], fx[:, sl], -1.0, 1.0, op.mult, op.add)
            tt(wx0[:, sl], t1[:, sl], t3[:, sl], op.mult)
            ts(t1[:, sl], x0[:, sl], -1.0, None, op.is_ge)
            ts(t2[:, sl], x0[:, sl], float(W - 2), None, op.is_le)
            tt(t3[:, sl], t1[:, sl], t2[:, sl], op.mult)
            tt(wx1[:, sl], fx[:, sl], t3[:, sl], op.mult)
            tt(wy0[:, sl], wy0[:, sl], mk_b, op.mult)
            tt(wy1[:, sl], wy1[:, sl], mk_b, op.mult)

            def w4_slot(j):
                return AP(w4.tensor, w4[:].offset + j + c0 * 4,
                          [w4[:].ap[0], [4, nq]])
            tt(w4_slot(0), wy0[:, sl], wx0[:, sl], op.mult)
            tt(w4_slot(1), wy0[:, sl], wx1[:, sl], op.mult)
            tt(w4_slot(2), wy1[:, sl], wx0[:, sl], op.mult)
            tt(w4_slot(3), wy1[:, sl], wx1[:, sl], op.mult)

            # gather index = table row r = clip(LEAD + y0*W + x0, 0, RQ-1)
            nc.vector.scalar_tensor_tensor(idxf[:, sl], y0[:, sl], float(W),
                                           x0[:, sl], op.mult, op.add)
            ts(idxf[:, sl], idxf[:, sl], float(LEAD), 0.0, op.add, op.max)
            ts(idxf[:, sl], idxf[:, sl], float(RQ - 1), None, op.min)
            nc.vector.tensor_copy(idx_t[:, sl], idxf[:, sl])

            # 16-wrap via DRAM round-trip fold + in-partition interleave;
            # queue 0 reads idx partitions 0..31 only -> replicate once.
            wr = nc.sync.dma_start(
                out=AP(idx_dram, c0, [[QTOT, 128], [1, nq]]),
                in_=idx_t[:, sl])
            rdf = nc.sync.dma_start(
                out=AP(fold.tensor, fold[:].offset + c0,
                       [fold[:].ap[0], [QTOT, 8], [1, nq]]),
                in_=AP(idx_dram, c0, [[QTOT, 16], [16 * QTOT, 8], [1, nq]]))
            add_dep_helper(rdf.ins, wr.ins, sync=False, reason="idx fold rt")
            idxw = cp.tile([128, nq * 8], dt.int16, tag=f"idxw{g0}")
            idxw_tiles[g0] = (g0, gn, idxw)
            i16 = idxw[0:16, :]
            nc.vector.tensor_copy(
                AP(i16.tensor, i16.offset,
                   [i16.ap[0], [1, 8], [8, nq]]),
                AP(fold.tensor, fold[:].offset + c0,
                   [fold[:].ap[0], [QTOT, 8], [1, nq]]))
            nc.sync.dma_start(out=idxw[16:32, :], in_=idxw[0:16, :])

        # ---------------- emission schedule ----------------
        # Everything ordered so gather 0's deps (band 0 + group 0 idx) clear
        # as early as possible on each in-order engine queue.
        for g in range(3):
            emit_trans(g)                      # x_hwc chunks 0..12
        emit_band(0)
        conv_tile(*trows[0]); conv_tile(*trows[1]); conv_tile(*trows[2])
        tcols_group(0)
        emit_group(0, 4)
        emit_group(4, 4)
        scale_t, shift_t = emit_bn()
        for g in range(3, 8):
            emit_trans(g)                      # chunks 12..32
        emit_band(1)
        conv_tile(*trows[3]); conv_tile(*trows[4])
        tcols_group(1)
        emit_group(8, 8)
        for g in range(8, 14):
            emit_trans(g)                      # chunks 32..54
        emit_band(2)
        for t_ in range(5, 9):
            conv_tile(*trows[t_])
        tcols_group(2); tcols_group(3)
        emit_group(16, 16)
        for t_ in range(9, 14):
            conv_tile(*trows[t_])
        for g in range(4, 7):
            tcols_group(g)
        emit_group(32, 22)

        # ---------------- gather / combine / matmul / epilogue -------------
        for si, (b0, nb) in enumerate(_slices()):
            Q = nb * 9
            npx = nb * 128
            q0 = b0 * 9
            gq = wp.tile([128, SLICE_BLOCKS * 9 * QW], dt.bfloat16, tag="gq",
                         bufs=3)
            gq3 = AP(gq.tensor, gq[:].offset,
                     [gq[:].ap[0], [QW, Q], [1, QW]])
            grp = next(v for v in idxw_tiles.values()
                       if v[0] <= b0 and b0 + nb <= v[0] + v[1])
            lq = (b0 - grp[0]) * 9
            gi = nc.gpsimd.dma_gather(
                out_ap=gq3,
                in_ap=x_quad[:],
                idxs_ap=grp[2][:, lq * 8:(lq + Q) * 8],
                num_idxs=Q * 128,
                num_idxs_reg=Q * 128,
                elem_size=QW,
                single_packet=False,
            )
            for dep in _slice_table_deps(b0, nb):
                add_dep_helper(gi.ins, dep.ins, reason="gather after table")

            # weighted 4-corner combine (in-place over the gathered tile;
            # slot pads multiply garbage that is never read)
            prod = gq
            gview = gq[:, :Q * QW]
            wb = AP(w4.tensor, w4[:].offset + q0 * 4,
                    [w4[:].ap[0], [1, Q * 4], [0, 64]])
            nc.vector.tensor_tensor(gview, gview, wb, op.mult)

            def pview(off):
                return AP(prod.tensor, prod[:].offset + off,
                          [prod[:].ap[0], [QW, Q], [1, C]])
            sa = wp.tile([128, SLICE_BLOCKS * 9 * C], dt.bfloat16, tag="sa")
            # sampt doubles as scratch for the second corner-pair sum; its
            # cols are rewritten only after the final add consumed them
            sampt = wp.tile([128, 4 * SLICE_BLOCKS * 128], dt.bfloat16, tag="sampt")
            sb2 = sampt
            nc.vector.tensor_tensor(sa[:, :Q * C], pview(0), pview(64), op.add)
            nc.vector.tensor_tensor(sb2[:, :Q * C], pview(128), pview(192),
                                    op.add)
            samp = sa
            nc.vector.tensor_tensor(samp[:, :Q * C], sa[:, :Q * C],
                                    sb2[:, :Q * C], op.add)
            for ib in range(nb):
                ps = pp_st.tile([128, 512], dt.bfloat16, tag="ps_st")
                base = ib * 9 * C
                for ch in range(3):
                    nc.tensor.transpose(
                        ps[:, ch * 128:(ch + 1) * 128],
                        samp[:, base + ch * 128: base + (ch + 1) * 128],
                        id128[:])
                nc.tensor.transpose(ps[0:C, 384:512],
                                    samp[:, base + 384: base + 432], id128[:])
                dst02 = AP(sampt.tensor, sampt[:].offset + ib * 128,
                           [sampt[:].ap[0], [npx, 3], [1, 128]])
                nc.scalar.activation(dst02, ps[:, 0:384], act.Copy)
                nc.scalar.activation(sampt[0:C, 3 * npx + ib * 128:
                                           3 * npx + (ib + 1) * 128],
                                     ps[0:C, 384:512], act.Copy)

            # matmul: out[o, px] += dwT_chunk.T @ sampt_chunk
            pso = pp_out.tile([C, SLICE_BLOCKS * 128], dt.float32, tag="ps_out")
            for ch in range(3):
                nc.tensor.matmul(pso[:, :npx], dwt_ap(ch, 128),
                                 sampt[:, ch * npx:(ch + 1) * npx],
                                 start=(ch == 0), stop=False)
            nc.tensor.matmul(pso[:, :npx], dwt_ap(3, C),
                             sampt[0:C, 3 * npx:3 * npx + npx],
                             start=False, stop=True)

            # epilogue: BN(running stats) + residual (DVE) + relu (ACT)
            bno = wp.tile([C, SLICE_BLOCKS * 128], dt.float32, tag="bno")
            nc.scalar.activation(bno[:, :npx], pso[:, :npx], act.Identity,
                                 bias=shift_t[:], scale=scale_t[:])
            nc.vector.tensor_tensor(bno[:, :npx], bno[:, :npx],
                                    x_sb[:, b0 * 128:b0 * 128 + npx], op.add)
            bno2 = wp.tile([C, SLICE_BLOCKS * 128], dt.float32, tag="bno2")
            nc.scalar.activation(bno2[:, :npx], bno[:, :npx], act.Relu)
            nc.sync.dma_start(out_ext[:, b0 * 128:b0 * 128 + npx],
                              bno2[:, :npx])

    nc.compile()
    return nc


def _host_pack(inputs):
    """Weight/constant layout prep (no input-data compute)."""
    bf16 = ml_dtypes.bfloat16
    wa = np.concatenate([inputs['tm_w'], inputs['tr_w'], inputs['mk_w']],
                        axis=0)                       # [15, C, 3, 3]
    wconv = np.ascontiguousarray(
        wa.transpose(1, 2, 3, 0).reshape(C, 9 * 15)).astype(bf16)
    bconv = np.concatenate([inputs['tm_b'], inputs['tr_b'],
                            inputs['mk_b']]).astype(np.float32)
    dwr = inputs['dw'].reshape(C, C, 9)               # [o, c, k]
    dwT = np.zeros((512, C), dtype=np.float32)
    dwT[:432] = dwr.transpose(2, 1, 0).reshape(9 * C, C)
    dwt = np.ascontiguousarray(
        dwT.reshape(4, 128, C).transpose(1, 0, 2).reshape(128, 4 * C)
    ).astype(bf16)

    blob16 = np.zeros((128, 327), dtype=bf16)
    blob16[:C, 0:135] = wconv
    blob16[:, 135:327] = dwt

    pix = (np.arange(NB)[None, :] * 128 + np.arange(128)[:, None])
    blob32 = np.zeros((128, 131), dtype=np.float32)
    blob32[:, 0:9] = np.tile(_REG[0], (128, 1))
    blob32[:, 9:18] = np.tile(_REG[1], (128, 1))
    blob32[:, 18:72] = (pix // W).astype(np.float32)
    blob32[:, 72:126] = (pix % W).astype(np.float32)
    blob32[:15, 126] = bconv
    blob32[:C, 127] = inputs['gamma'].astype(np.float32)
    blob32[:C, 128] = inputs['beta'].astype(np.float32)
    blob32[:C, 129] = inputs['rmean'].astype(np.float32)
    blob32[:C, 130] = inputs['rvar'].astype(np.float32)
    return dict(blob16=blob16, blob32=blob32)


def kernel(**inputs):
    inputs = {k: np.asarray(v) for k, v in inputs.items()}
    if 'nc' not in _built:
        _built['nc'] = build_nc()
    nc = _built['nc']

    from concourse.bass_utils import run_bass_kernel_spmd
    shared = _host_pack(inputs)
    x = inputs['x'].astype(np.float32)
    in_maps = []
    for i in range(N):
        m = dict(shared)
        m['x'] = np.ascontiguousarray(x[i].reshape(C, HW))
        in_maps.append(m)
    res = run_bass_kernel_spmd(nc, in_maps, core_ids=list(range(N)))
    out = np.stack([res.results[i]['out'].reshape(C, H, W)
                    for i in range(N)])
    return out.astype(np.float32)


# revision 6
# speedup vs baseline: 1.1465x; 1.1250x over previous
"""Trainium2 Bass kernel for nn_AdaptBlockV2 (deformable-conv-v2 block).

Data-parallel over the batch axis: 8 samples -> 8 NeuronCores, one sample
per core. Inside each core:
  A) load x; build zero-padded CHW copy (bf16) for the convs; transpose x to
     HWC (bf16, three band tiles) and write a ROW-MAJOR "quad" gather table
     straight to DRAM with 4 strided DMAs per band (row r = channels of flat
     pixels [r, r+1, r+W, r+W+1] at slots 0..3) -- one indirect-DMA
     descriptor then fetches all 4 bilinear corners of one (pixel, tap).
     Table edge rows are pre-zeroed so zero-weight fetches read 0, not NaN.
  B) 15-channel 3x3 conv (offset transform T, translation tr, modulation
     mask) as 9 PSUM-accumulated matmuls; transpose conv output to
     pixel-major; bulk DVE math for sampling positions py/px, floor via
     floored-mod, corner weights (bilinear x mask x validity), and the flat
     gather index (= table row, no remap needed with the row-major table).
  C) per-slice pipeline: indirect DMA gather -> DVE weighted 4-corner
     combine -> PE transpose of samp to (tap,channel)-major -> matmul with
     dw -> BN (running stats) + residual (DVE add) + ReLU (Scalar engine)
     -> DMA out.

The SWDGE gather descriptor generation on the Pool engine (~8ns/descriptor
x 62208 descriptors ~= 490us) is the hard floor; the prologue is kept to
~25us by minimizing serial Sync-engine DMA issues (the quad table is 10
DMAs instead of ~72; the idx 16-wrap fold is 3 DMAs per group instead of 9
-- queue 0 only needs the indices replicated to partitions 0..31).

kernel(**inputs) takes FULL unsharded inputs, returns the FULL output.
"""
import numpy as np
import ml_dtypes

N, C, H, W = 8, 48, 96, 72
HW = H * W                       # 6912
LEAD = W + 2                     # 74: lead pad rows in the quad table
RQ = 7040                        # quad-table rows (>= HW + W + 2)
QW = 256                         # quad-table row width (512B, dma_gather)
NB = HW // 128                   # 54 pixel blocks
QTOT = NB * 9                    # 486 (block, tap) chunks
PADW = W + 2                     # 74 padded conv row stride
PADLEN = (H + 2) * PADW         # 7252
BN_EPS = 1e-5
CONV_ROWS = 7                    # conv N-tile = 7 image rows = 504 pixels
SLICE_BLOCKS = 4                 # gather/combine slice = 4 pixel blocks
MAXD = 10                        # sample-displacement bound for band deps
BANDS = [(0, 12), (12, 32), (32, 54)]   # x_hwc chunk ranges per table band
DELTA = [0, 1, W, W + 1]                # quad slot pixel offsets

_REG = np.array([[-1, -1, -1, 0, 0, 0, 1, 1, 1],
                 [-1, 0, 1, -1, 0, 1, -1, 0, 1]], dtype=np.float32)

_built = {}


def _slices():
    out = []
    b = 0
    while b < NB:
        nb = min(SLICE_BLOCKS, NB - b)
        out.append((b, nb))
        b += nb
    return out


def build_nc():
    import concourse.bass as bass
    import concourse.bacc as bacc
    import concourse.tile as tile
    from concourse import mybir
    from concourse.bass import AP
    from concourse.masks import make_identity
    from concourse.tile import add_dep_helper
    from contextlib import ExitStack

    dt = mybir.dt
    op = mybir.AluOpType
    act = mybir.ActivationFunctionType

    nc = bacc.Bacc("TRN2", target_bir_lowering=False, debug=False,
                   num_devices=N, dynamic_dma_scratch_size=16384,
                   num_swdge_queues=4)
    x_ext = nc.declare_dram_parameter("x", [C, HW], dt.float32, isOutput=False)
    blob16_ext = nc.declare_dram_parameter("blob16", [128, 327], dt.bfloat16,
                                           isOutput=False)
    blob32_ext = nc.declare_dram_parameter("blob32", [128, 131], dt.float32,
                                           isOutput=False)
    out_ext = nc.declare_dram_parameter("out", [C, HW], dt.float32, isOutput=True)

    x_quad = nc.dram_tensor("x_quad", [RQ, QW], dt.bfloat16)
    idx_dram = nc.dram_tensor("idx_dram", [128 * QTOT], dt.int16)

    with tile.TileContext(nc) as tc, ExitStack() as ctx:
        cp = ctx.enter_context(tc.tile_pool(name="const", bufs=1))
        tp = ctx.enter_context(tc.tile_pool(name="tmp", bufs=1))
        wp = ctx.enter_context(tc.tile_pool(name="work", bufs=2))
        pp_a = ctx.enter_context(tc.tile_pool(name="ps_a", bufs=2, space="PSUM"))
        pp_st = ctx.enter_context(tc.tile_pool(name="ps_st", bufs=2, space="PSUM"))
        pp_out = ctx.enter_context(tc.tile_pool(name="ps_out", bufs=1, space="PSUM"))

        # ---------------- constants / weights to SBUF ----------------
        x_sb = cp.tile([C, HW], dt.float32, tag="x_sb")
        nc.sync.dma_start(x_sb[:], x_ext[:])
        blob16 = cp.tile([128, 327], dt.bfloat16, tag="blob16")
        nc.sync.dma_start(blob16[:], blob16_ext[:])
        blob32 = cp.tile([128, 131], dt.float32, tag="blob32")
        nc.sync.dma_start(blob32[:], blob32_ext[:])

        b16p = blob16[:].ap[0][0]
        b16o = blob16[:].offset
        b32p = blob32[:].ap[0][0]
        b32o = blob32[:].offset

        def wconv_ap(s):         # [48, 15] stationary for conv tap s
            return AP(blob16.tensor, b16o + s * 15, [[b16p, C], [1, 15]])

        def dwt_ap(ch, nparts):  # [nparts, 48] stationary for dw chunk ch
            return AP(blob16.tensor, b16o + 135 + ch * C, [[b16p, nparts], [1, C]])

        def b32col(col, nparts):
            return AP(blob32.tensor, b32o + col, [[b32p, nparts], [1, 1]])

        bconv_ap = b32col(126, 15)

        id48 = cp.tile([C, C], dt.float32, tag="id48")
        make_identity(nc, id48[:])
        id16 = id48[0:15, 0:15]
        id128 = cp.tile([128, 128], dt.bfloat16, tag="id128")
        make_identity(nc, id128[:])

        # Dummy 128-descriptor gather issued first: forces the Pool engine's
        # gather-ucode LOAD_LIB (which barriers on all previously emitted
        # work) to happen immediately against an empty pipeline instead of
        # right before the first real gather.
        dz_idx = tp.tile([128, 8], dt.int16, tag="dz_idx")
        nc.gpsimd.memset(dz_idx[:], 0)
        dz_out = tp.tile([128, 256], dt.bfloat16, tag="dz_out")
        nc.gpsimd.dma_gather(
            out_ap=AP(dz_out.tensor, dz_out[:].offset,
                      [dz_out[:].ap[0], [QW, 1], [1, QW]]),
            in_ap=x_quad[:], idxs_ap=dz_idx[:],
            num_idxs=128, num_idxs_reg=128,
            elem_size=QW, single_packet=False)

        # ---------------- zero the quad-table edge rows ----------------
        # Rows [0, LEAD) and [RQ-127, RQ) can be fetched (clamped idx /
        # out-of-image corners) with zero weight; they must hold 0, not junk.
        ztile = tp.tile([128, 254], dt.bfloat16, tag="ztile")
        nc.vector.memset(ztile[:], 0.0)
        zlow = nc.sync.dma_start(
            out=AP(x_quad, 0, [[148, 128], [1, 148]]),
            in_=ztile[:, :148])
        zhigh = nc.sync.dma_start(
            out=AP(x_quad, (RQ - 127) * QW, [[254, 128], [1, 254]]),
            in_=ztile[:, :254])

        # ---------------- padded CHW copy (bf16) for convs ----------------
        # Only the pad cells are memset (head row+corner, right|left pad
        # pairs, tail), then the interior is cast-copied from x_sb.
        x_pad = cp.tile([C, PADLEN], dt.bfloat16, tag="x_pad")
        nc.vector.memset(x_pad[:, 0:PADW + 1], 0.0)
        nc.vector.memset(
            AP(x_pad.tensor, x_pad[:].offset + 2 * PADW - 1,
               [x_pad[:].ap[0], [PADW, H - 1], [1, 2]]), 0.0)
        nc.vector.memset(x_pad[:, (H + 1) * PADW - 1:PADLEN], 0.0)
        xpad_int = AP(x_pad.tensor, x_pad[:].offset + PADW + 1,
                      [x_pad[:].ap[0], [PADW, H], [1, W]])
        nc.vector.tensor_copy(xpad_int, x_sb[:])   # f32 -> bf16 cast on DVE

        # ---------------- x -> HWC (bf16) via PE transposes ----------------
        # One tile per table band so band writes only dep on their own chunks.
        hwc_tiles = [cp.tile([128, (k1 - k0) * C], dt.bfloat16,
                             name=f"x_hwc{i}", tag=f"x_hwc{i}")
                     for i, (k0, k1) in enumerate(BANDS)]

        def hwc_of(chunk):
            for (k0, k1), t in zip(BANDS, hwc_tiles):
                if k0 <= chunk < k1:
                    return t, k0
            raise AssertionError

        def emit_trans(g):                    # 4 blocks per PSUM tile
            nblk = min(4, NB - g * 4)
            ps = pp_a.tile([128, 4 * C], dt.float32, name="psA", tag="psA")
            for j in range(nblk):
                b = g * 4 + j
                nc.tensor.transpose(ps[:, j * C:(j + 1) * C],
                                    x_sb[:, b * 128:(b + 1) * 128], id48[:])
            t, k0 = hwc_of(g * 4)
            nc.scalar.activation(
                t[:, (g * 4 - k0) * C:(g * 4 - k0 + nblk) * C],
                ps[:, :nblk * C], act.Copy)

        # ---------------- quad table: 4 slot DMAs per band -----------------
        band_ins = {}

        def emit_band(bi):
            k0, k1 = BANDS[bi]
            nk = k1 - k0
            t, tk0 = hwc_of(k0)
            ws = []
            for j, dj in enumerate(DELTA):
                base = (LEAD - dj) * QW + 64 * j
                w = nc.sync.dma_start(
                    out=AP(x_quad, base + k0 * 128 * QW,
                           [[QW, 128], [128 * QW, nk], [1, C]]),
                    in_=AP(t.tensor, t[:].offset + (k0 - tk0) * C,
                           [t[:].ap[0], [C, nk], [1, C]]))
                if bi == 0:
                    add_dep_helper(w.ins, zlow.ins, reason="slot after zero")
                if bi == 2:
                    add_dep_helper(w.ins, zhigh.ins, reason="slot after zero")
                ws.append(w)
            band_ins[bi] = ws

        def _slice_table_deps(b0, nb):
            y_lo = (b0 * 128) // W
            y_hi = ((b0 + nb) * 128 - 1) // W
            r_lo = max(0, LEAD + (y_lo - MAXD) * W)
            r_hi = min(RQ - 1, LEAD + (y_hi + MAXD + 1) * W + W - 1)
            deps = []
            if r_lo < LEAD:
                deps.append(zlow)
            if r_hi >= RQ - 127:
                deps.append(zhigh)
            for bi, (k0, k1) in enumerate(BANDS):
                if r_lo <= k1 * 128 + W + 1 and r_hi >= k0 * 128 + 1:
                    deps.extend(band_ins[bi])
            return deps

        # bn scale' = gamma * rsqrt(rvar+eps); shift' = beta - rmean*scale'
        def emit_bn():
            veps = tp.tile([C, 1], dt.float32, tag="veps")
            nc.vector.tensor_scalar(veps[:], b32col(130, C), BN_EPS, None, op.add)
            vsq = tp.tile([C, 1], dt.float32, tag="vsq")
            nc.scalar.activation(vsq[:], veps[:], act.Sqrt)
            vri = tp.tile([C, 1], dt.float32, tag="vri")
            nc.vector.reciprocal(vri[:], vsq[:])
            scale_t = cp.tile([C, 1], dt.float32, tag="scale")
            nc.vector.tensor_tensor(scale_t[:], b32col(127, C), vri[:], op.mult)
            vms = tp.tile([C, 1], dt.float32, tag="vms")
            nc.vector.tensor_tensor(vms[:], b32col(129, C), scale_t[:], op.mult)
            shift_t = cp.tile([C, 1], dt.float32, tag="shift")
            nc.vector.tensor_tensor(shift_t[:], b32col(128, C), vms[:], op.subtract)
            return scale_t, shift_t

        # ---------------- convs: 15ch 3x3 via 9 accumulated matmuls --------
        conv_sb = cp.tile([15, HW], dt.float32, tag="conv_sb")
        trows = [(t * CONV_ROWS, min(CONV_ROWS, H - t * CONV_ROWS))
                 for t in range((H + CONV_ROWS - 1) // CONV_ROWS)]

        def conv_tile(r0, nr):
            psc = pp_a.tile([15, CONV_ROWS * W], dt.float32, name="psA2",
                            tag="psA2", bufs=3)
            npx = nr * W
            for s in range(9):
                r, c = divmod(s, 3)
                rhs = AP(x_pad.tensor,
                         x_pad[:].offset + (r0 + r) * PADW + c,
                         [x_pad[:].ap[0], [PADW, nr], [1, W]])
                nc.tensor.matmul(psc[:, :npx], wconv_ap(s),
                                 rhs, start=(s == 0), stop=(s == 8))
            nc.scalar.activation(conv_sb[:, r0 * W:r0 * W + npx], psc[:, :npx],
                                 act.Identity, bias=bconv_ap)

        # conv output -> pixel-major (tcols), per 8-block group
        tcols = cp.tile([128, NB * 15], dt.float32, tag="tcols")

        def tcols_group(g):
            nblk = min(8, NB - g * 8)
            ps = pp_a.tile([128, 8 * 15], dt.float32, name="psA", tag="psA")
            for j in range(nblk):
                b = g * 8 + j
                nc.tensor.transpose(ps[:, j * 15:(j + 1) * 15],
                                    conv_sb[:, b * 128:(b + 1) * 128],
                                    id16[:])
            nc.scalar.activation(tcols[:, g * 8 * 15:(g * 8 + nblk) * 15],
                                 ps[:, :nblk * 15], act.Copy)

        # ---------------- bulk offset / weight / index math ----------------
        def plane(tag):
            return tp.tile([128, QTOT], dt.float32, name=tag, tag=tag)

        py = plane("py"); px = plane("px")
        t1 = plane("t1"); t2 = plane("t2"); t3 = plane("t3")
        fy = plane("fy"); fx = plane("fx")
        y0 = plane("y0"); x0 = plane("x0")
        wy0 = py; wy1 = px                       # reuse dead slots (disjoint cols)
        wx0 = plane("wx0"); wx1 = plane("wx1")
        idxf = fy                                # reuse dead slot
        cvt_i = tp.tile([128, QTOT], dt.int32, name="cvt_i", tag="cvt_i")
        w4 = cp.tile([128, QTOT * 4], dt.bfloat16, tag="w4")
        idx_t = cp.tile([128, QTOT], dt.int16, tag="idx")
        fold = cp.tile([16, 8 * QTOT], dt.int16, tag="fold")
        # one idxw tile per offset-math group, so a slice gather depends only
        # on its own group's fold (Tile tracks whole-tile versions)
        idxw_tiles = {}
        groups = [(0, 4), (4, 4), (8, 8), (16, 16), (32, 22)]

        def emit_group(g0, gn):
            nq = gn * 9
            c0 = g0 * 9
            sl = slice(c0, c0 + nq)

            def tcol_b(ch):
                return AP(tcols.tensor, tcols[:].offset + ch + g0 * 15,
                          [tcols[:].ap[0], [15, gn], [0, 9]])

            def reg_b(roff):
                return AP(blob32.tensor, b32o + roff,
                          [[b32p, 128], [0, gn], [1, 9]])

            def coord_b(coff):
                return AP(blob32.tensor, b32o + coff + g0,
                          [[b32p, 128], [1, gn], [0, 9]])

            tt = nc.vector.tensor_tensor
            ts = nc.vector.tensor_scalar
            tt(t1[:, sl], reg_b(0), tcol_b(0), op.mult)
            tt(t2[:, sl], reg_b(9), tcol_b(1), op.mult)
            tt(t3[:, sl], t1[:, sl], t2[:, sl], op.add)
            tt(t1[:, sl], t3[:, sl], tcol_b(4), op.add)
            tt(py[:, sl], t1[:, sl], coord_b(18), op.add)
            tt(t1[:, sl], reg_b(0), tcol_b(2), op.mult)
            tt(t2[:, sl], reg_b(9), tcol_b(3), op.mult)
            tt(t3[:, sl], t1[:, sl], t2[:, sl], op.add)
            tt(t1[:, sl], t3[:, sl], tcol_b(5), op.add)
            tt(px[:, sl], t1[:, sl], coord_b(72), op.add)

            # floor(v) = int(v) - (v < int(v)); rounding-mode agnostic
            for (v, fl, fr) in ((py, y0, fy), (px, x0, fx)):
                nc.vector.tensor_copy(cvt_i[:, sl], v[:, sl])
                nc.vector.tensor_copy(t1[:, sl], cvt_i[:, sl])
                tt(t2[:, sl], v[:, sl], t1[:, sl], op.is_lt)
                tt(fl[:, sl], t1[:, sl], t2[:, sl], op.subtract)
                tt(fr[:, sl], v[:, sl], fl[:, sl], op.subtract)

            mk_b = AP(tcols.tensor, tcols[:].offset + 6 + g0 * 15,
                      [tcols[:].ap[0], [15, gn], [1, 9]])
            ts(t1[:, sl], y0[:, sl], 0.0, None, op.is_ge)
            ts(t2[:, sl], y0[:, sl], float(H - 1), None, op.is_le)
            tt(t3[:, sl], t1[:, sl], t2[:, sl], op.mult)
            ts(t1[:, sl], fy[:, sl], -1.0, 1.0, op.mult, op.add)
            tt(wy0[:, sl], t1[:, sl], t3[:, sl], op.mult)
            ts(t1[:, sl], y0[:, sl], -1.0, None, op.is_ge)
            ts(t2[:, sl], y0[:, sl], float(H - 2), None, op.is_le)
            tt(t3[:, sl], t1[:, sl], t2[:, sl], op.mult)
            tt(wy1[:, sl], fy[:, sl], t3[:, sl], op.mult)
            ts(t1[:, sl], x0[:, sl], 0.0, None, op.is_ge)
            ts(t2[:, sl], x0[:, sl], float(W - 1), None, op.is_le)
            tt(t3[:, sl], t1[:, sl], t2[:, sl], op.mult)
            ts(t1[:, sl], fx[:, sl], -1.0, 1.0, op.mult, op.add)
            tt(wx0[:, sl], t1[:, sl], t3[:, sl], op.mult)
            ts(t1[:, sl], x0[:, sl], -1.0, None, op.is_ge)
            ts(t2[:, sl], x0[:, sl], float(W - 2), None, op.is_le)
            tt(t3[:, sl], t1[:, sl], t2[:, sl], op.mult)
            tt(wx1[:, sl], fx[:, sl], t3[:, sl], op.mult)
            tt(wy0[:, sl], wy0[:, sl], mk_b, op.mult)
            tt(wy1[:, sl], wy1[:, sl], mk_b, op.mult)

            def w4_slot(j):
                return AP(w4.tensor, w4[:].offset + j + c0 * 4,
                          [w4[:].ap[0], [4, nq]])
            tt(w4_slot(0), wy0[:, sl], wx0[:, sl], op.mult)
            tt(w4_slot(1), wy0[:, sl], wx1[:, sl], op.mult)
            tt(w4_slot(2), wy1[:, sl], wx0[:, sl], op.mult)
            tt(w4_slot(3), wy1[:, sl], wx1[:, sl], op.mult)

            # gather index = table row r = clip(LEAD + y0*W + x0, 0, RQ-1)
            nc.vector.scalar_tensor_tensor(idxf[:, sl], y0[:, sl], float(W),
                                           x0[:, sl], op.mult, op.add)
            ts(idxf[:, sl], idxf[:, sl], float(LEAD), 0.0, op.add, op.max)
            ts(idxf[:, sl], idxf[:, sl], float(RQ - 1), None, op.min)
            nc.vector.tensor_copy(idx_t[:, sl], idxf[:, sl])

            # 16-wrap via DRAM round-trip fold + in-partition interleave;
            # queue 0 reads idx partitions 0..31 only -> replicate once.
            wr = nc.sync.dma_start(
                out=AP(idx_dram, c0, [[QTOT, 128], [1, nq]]),
                in_=idx_t[:, sl])
            rdf = nc.sync.dma_start(
                out=AP(fold.tensor, fold[:].offset + c0,
                       [fold[:].ap[0], [QTOT, 8], [1, nq]]),
                in_=AP(idx_dram, c0, [[QTOT, 16], [16 * QTOT, 8], [1, nq]]))
            add_dep_helper(rdf.ins, wr.ins, sync=False, reason="idx fold rt")
            idxw = cp.tile([128, nq * 8], dt.int16, tag=f"idxw{g0}")
            idxw_tiles[g0] = (g0, gn, idxw)
            i16 = idxw[0:16, :]
            nc.vector.tensor_copy(
                AP(i16.tensor, i16.offset,
                   [i16.ap[0], [1, 8], [8, nq]]),
                AP(fold.tensor, fold[:].offset + c0,
                   [fold[:].ap[0], [QTOT, 8], [1, nq]]))
            nc.sync.dma_start(out=idxw[16:32, :], in_=idxw[0:16, :])
            nc.sync.dma_start(out=idxw[32:64, :], in_=idxw[0:32, :])
            nc.sync.dma_start(out=idxw[64:128, :], in_=idxw[0:64, :])

        # ---------------- emission schedule ----------------
        # Everything ordered so gather 0's deps (band 0 + group 0 idx) clear
        # as early as possible on each in-order engine queue.
        for g in range(3):
            emit_trans(g)                      # x_hwc chunks 0..12
        emit_band(0)
        conv_tile(*trows[0]); conv_tile(*trows[1]); conv_tile(*trows[2])
        tcols_group(0)
        emit_group(0, 4)
        emit_group(4, 4)
        scale_t, shift_t = emit_bn()
        for g in range(3, 8):
            emit_trans(g)                      # chunks 12..32
        emit_band(1)
        conv_tile(*trows[3]); conv_tile(*trows[4])
        tcols_group(1)
        emit_group(8, 8)
        for g in range(8, 14):
            emit_trans(g)                      # chunks 32..54
        emit_band(2)
        for t_ in range(5, 9):
            conv_tile(*trows[t_])
        tcols_group(2); tcols_group(3)
        emit_group(16, 16)
        for t_ in range(9, 14):
            conv_tile(*trows[t_])
        for g in range(4, 7):
            tcols_group(g)
        emit_group(32, 22)

        # ---------------- gather / combine / matmul / epilogue -------------
        for si, (b0, nb) in enumerate(_slices()):
            Q = nb * 9
            npx = nb * 128
            q0 = b0 * 9
            gq = wp.tile([128, SLICE_BLOCKS * 9 * QW], dt.bfloat16, tag="gq",
                         bufs=3)
            gq3 = AP(gq.tensor, gq[:].offset,
                     [gq[:].ap[0], [QW, Q], [1, QW]])
            grp = next(v for v in idxw_tiles.values()
                       if v[0] <= b0 and b0 + nb <= v[0] + v[1])
            lq = (b0 - grp[0]) * 9
            gi = nc.gpsimd.dma_gather(
                out_ap=gq3,
                in_ap=x_quad[:],
                idxs_ap=grp[2][:, lq * 8:(lq + Q) * 8],
                num_idxs=Q * 128,
                num_idxs_reg=Q * 128,
                elem_size=QW,
                single_packet=False,
                queue_num=si % 4,
            )
            for dep in _slice_table_deps(b0, nb):
                add_dep_helper(gi.ins, dep.ins, reason="gather after table")

            # weighted 4-corner combine (in-place over the gathered tile;
            # slot pads multiply garbage that is never read)
            prod = gq
            gview = gq[:, :Q * QW]
            wb = AP(w4.tensor, w4[:].offset + q0 * 4,
                    [w4[:].ap[0], [1, Q * 4], [0, 64]])
            nc.vector.tensor_tensor(gview, gview, wb, op.mult)

            def pview(off):
                return AP(prod.tensor, prod[:].offset + off,
                          [prod[:].ap[0], [QW, Q], [1, C]])
            sa = wp.tile([128, SLICE_BLOCKS * 9 * C], dt.bfloat16, tag="sa")
            # sampt doubles as scratch for the second corner-pair sum; its
            # cols are rewritten only after the final add consumed them
            sampt = wp.tile([128, 4 * SLICE_BLOCKS * 128], dt.bfloat16, tag="sampt")
            sb2 = sampt
            nc.vector.tensor_tensor(sa[:, :Q * C], pview(0), pview(64), op.add)
            nc.vector.tensor_tensor(sb2[:, :Q * C], pview(128), pview(192),
                                    op.add)
            samp = sa
            nc.vector.tensor_tensor(samp[:, :Q * C], sa[:, :Q * C],
                                    sb2[:, :Q * C], op.add)
            for ib in range(nb):
                ps = pp_st.tile([128, 512], dt.bfloat16, tag="ps_st")
                base = ib * 9 * C
                for ch in range(3):
                    nc.tensor.transpose(
                        ps[:, ch * 128:(ch + 1) * 128],
                        samp[:, base + ch * 128: base + (ch + 1) * 128],
                        id128[:])
                nc.tensor.transpose(ps[0:C, 384:512],
                                    samp[:, base + 384: base + 432], id128[:])
                dst02 = AP(sampt.tensor, sampt[:].offset + ib * 128,
                           [sampt[:].ap[0], [npx, 3], [1, 128]])
                nc.scalar.activation(dst02, ps[:, 0:384], act.Copy)
                nc.scalar.activation(sampt[0:C, 3 * npx + ib * 128:
                                           3 * npx + (ib + 1) * 128],
                                     ps[0:C, 384:512], act.Copy)

            # matmul: out[o, px] += dwT_chunk.T @ sampt_chunk
            pso = pp_out.tile([C, SLICE_BLOCKS * 128], dt.float32, tag="ps_out")
            for ch in range(3):
                nc.tensor.matmul(pso[:, :npx], dwt_ap(ch, 128),
                                 sampt[:, ch * npx:(ch + 1) * npx],
                                 start=(ch == 0), stop=False)
            nc.tensor.matmul(pso[:, :npx], dwt_ap(3, C),
                             sampt[0:C, 3 * npx:3 * npx + npx],
                             start=False, stop=True)

            # epilogue: BN(running stats) + residual (DVE) + relu (ACT)
            bno = wp.tile([C, SLICE_BLOCKS * 128], dt.float32, tag="bno")
            nc.scalar.activation(bno[:, :npx], pso[:, :npx], act.Identity,
                                 bias=shift_t[:], scale=scale_t[:])
            nc.vector.tensor_tensor(bno[:, :npx], bno[:, :npx],
                                    x_sb[:, b0 * 128:b0 * 128 + npx], op.add)
            bno2 = wp.tile([C, SLICE_BLOCKS * 128], dt.float32, tag="bno2")
            nc.scalar.activation(bno2[:, :npx], bno[:, :npx], act.Relu)
            nc.sync.dma_start(out_ext[:, b0 * 128:b0 * 128 + npx],
                              bno2[:, :npx])

    nc.compile()
    return nc


def _host_pack(inputs):
    """Weight/constant layout prep (no input-data compute)."""
    bf16 = ml_dtypes.bfloat16
    wa = np.concatenate([inputs['tm_w'], inputs['tr_w'], inputs['mk_w']],
                        axis=0)                       # [15, C, 3, 3]
    wconv = np.ascontiguousarray(
        wa.transpose(1, 2, 3, 0).reshape(C, 9 * 15)).astype(bf16)
    bconv = np.concatenate([inputs['tm_b'], inputs['tr_b'],
                            inputs['mk_b']]).astype(np.float32)
    dwr = inputs['dw'].reshape(C, C, 9)               # [o, c, k]
    dwT = np.zeros((512, C), dtype=np.float32)
    dwT[:432] = dwr.transpose(2, 1, 0).reshape(9 * C, C)
    dwt = np.ascontiguousarray(
        dwT.reshape(4, 128, C).transpose(1, 0, 2).reshape(128, 4 * C)
    ).astype(bf16)

    blob16 = np.zeros((128, 327), dtype=bf16)
    blob16[:C, 0:135] = wconv
    blob16[:, 135:327] = dwt

    pix = (np.arange(NB)[None, :] * 128 + np.arange(128)[:, None])
    blob32 = np.zeros((128, 131), dtype=np.float32)
    blob32[:, 0:9] = np.tile(_REG[0], (128, 1))
    blob32[:, 9:18] = np.tile(_REG[1], (128, 1))
    blob32[:, 18:72] = (pix // W).astype(np.float32)
    blob32[:, 72:126] = (pix % W).astype(np.float32)
    blob32[:15, 126] = bconv
    blob32[:C, 127] = inputs['gamma'].astype(np.float32)
    blob32[:C, 128] = inputs['beta'].astype(np.float32)
    blob32[:C, 129] = inputs['rmean'].astype(np.float32)
    blob32[:C, 130] = inputs['rvar'].astype(np.float32)
    return dict(blob16=blob16, blob32=blob32)


def kernel(**inputs):
    inputs = {k: np.asarray(v) for k, v in inputs.items()}
    if 'nc' not in _built:
        _built['nc'] = build_nc()
    nc = _built['nc']

    from concourse.bass_utils import run_bass_kernel_spmd
    shared = _host_pack(inputs)
    x = inputs['x'].astype(np.float32)
    in_maps = []
    for i in range(N):
        m = dict(shared)
        m['x'] = np.ascontiguousarray(x[i].reshape(C, HW))
        in_maps.append(m)
    res = run_bass_kernel_spmd(nc, in_maps, core_ids=list(range(N)))
    out = np.stack([res.results[i]['out'].reshape(C, H, W)
                    for i in range(N)])
    return out.astype(np.float32)


# revision 23
# speedup vs baseline: 1.5049x; 1.3126x over previous
"""Trainium2 Bass kernel for nn_AdaptBlockV2 (deformable-conv-v2 block).

Data-parallel over the batch axis: 8 samples -> 8 NeuronCores, one sample
per core. Inside each core:
  A) load x; build zero-padded CHW copy (bf16) for the convs; transpose x to
     HWC (bf16, three band tiles) and write a ROW-MAJOR "quad" gather table
     straight to DRAM with 4 strided DMAs per band (row r = channels of flat
     pixels [r, r+1, r+W, r+W+1] at slots 0..3) -- one indirect-DMA
     descriptor then fetches all 4 bilinear corners of one (pixel, tap).
     Table edge rows are pre-zeroed so zero-weight fetches read 0, not NaN.
  B) 15-channel 3x3 conv (offset transform T, translation tr, modulation
     mask) as 9 PSUM-accumulated matmuls; transpose conv output to
     pixel-major; bulk DVE math for sampling positions py/px, floor via
     floored-mod, corner weights (bilinear x mask x validity), and the flat
     gather index (= table row, no remap needed with the row-major table).
  C) per-slice pipeline: indirect DMA gather -> DVE weighted 4-corner
     combine -> PE transpose of samp to (tap,channel)-major -> matmul with
     dw -> BN (running stats) + residual (DVE add) + ReLU (Scalar engine)
     -> DMA out.

The SWDGE gather descriptor generation on the Pool engine (~8ns/descriptor
x 62208 descriptors ~= 490us) is the hard floor; the prologue is kept to
~25us by minimizing serial Sync-engine DMA issues (the quad table is 10
DMAs instead of ~72; the idx 16-wrap fold is 3 DMAs per group instead of 9
-- queue 0 only needs the indices replicated to partitions 0..31).

kernel(**inputs) takes FULL unsharded inputs, returns the FULL output.
"""
import numpy as np
import ml_dtypes

N, C, H, W = 8, 48, 96, 72
HW = H * W                       # 6912
LEAD = W + 2                     # 74: lead pad rows in the quad table
RQ = 7040                        # quad-table rows (>= HW + W + 2)
QW = 256                         # quad-table row width (512B, dma_gather)
NB = HW // 128                   # 54 pixel blocks
QTOT = NB * 9                    # 486 (block, tap) chunks
PADW = W + 2                     # 74 padded conv row stride
PADLEN = (H + 2) * PADW         # 7252
BN_EPS = 1e-5
CONV_ROWS = 7                    # conv N-tile = 7 image rows = 504 pixels
SLICE_BLOCKS = 4                 # gather/combine slice = 4 pixel blocks
MAXD = 10                        # sample-displacement bound for band deps
BANDS = [(0, 12), (12, 32), (32, 54)]   # x_hwc chunk ranges per table band
DELTA = [0, 1, W, W + 1]                # quad slot pixel offsets

_REG = np.array([[-1, -1, -1, 0, 0, 0, 1, 1, 1],
                 [-1, 0, 1, -1, 0, 1, -1, 0, 1]], dtype=np.float32)

_built = {}


def _slices():
    out = []
    b = 0
    while b < NB:
        nb = min(SLICE_BLOCKS, NB - b)
        out.append((b, nb))
        b += nb
    return out


def build_nc():
    import concourse.bass as bass
    import concourse.bacc as bacc
    import concourse.tile as tile
    from concourse import mybir
    from concourse.bass import AP
    from concourse.masks import make_identity
    from concourse.tile import add_dep_helper
    from contextlib import ExitStack

    dt = mybir.dt
    op = mybir.AluOpType
    act = mybir.ActivationFunctionType

    nc = bacc.Bacc("TRN2", target_bir_lowering=False, debug=False,
                   num_devices=N, dynamic_dma_scratch_size=16384,
                   num_swdge_queues=4)
    x_ext = nc.declare_dram_parameter("x", [C, HW], dt.bfloat16, isOutput=False)
    blob16_ext = nc.declare_dram_parameter("blob16", [128, 282], dt.bfloat16,
                                           isOutput=False)
    blob32_ext = nc.declare_dram_parameter("blob32", [128, 131], dt.float32,
                                           isOutput=False)
    out_ext = nc.declare_dram_parameter("out", [C, HW], dt.float32, isOutput=True)

    x_quad = nc.dram_tensor("x_quad", [RQ, QW], dt.bfloat16)
    idx_dram = nc.dram_tensor("idx_dram", [128 * QTOT], dt.int16)

    with tile.TileContext(nc) as tc, ExitStack() as ctx:
        cp = ctx.enter_context(tc.tile_pool(name="const", bufs=1))
        tp = ctx.enter_context(tc.tile_pool(name="tmp", bufs=1))
        wp = ctx.enter_context(tc.tile_pool(name="work", bufs=2))
        pp_a = ctx.enter_context(tc.tile_pool(name="ps_a", bufs=2, space="PSUM"))
        pp_st = ctx.enter_context(tc.tile_pool(name="ps_st", bufs=2, space="PSUM"))
        pp_out = ctx.enter_context(tc.tile_pool(name="ps_out", bufs=1, space="PSUM"))

        # ---------------- constants / weights to SBUF ----------------
        x_sb = cp.tile([C, HW], dt.bfloat16, tag="x_sb")
        nc.sync.dma_start(x_sb[:], x_ext[:])
        blob16 = cp.tile([128, 282], dt.bfloat16, tag="blob16")
        nc.sync.dma_start(blob16[:], blob16_ext[:])
        blob32 = cp.tile([128, 131], dt.float32, tag="blob32")
        nc.sync.dma_start(blob32[:], blob32_ext[:])

        b16p = blob16[:].ap[0][0]
        b16o = blob16[:].offset
        b32p = blob32[:].ap[0][0]
        b32o = blob32[:].offset

        def wconvA_ap(r):        # [96, 15] stationary: taps (r,0) + (r,1)
            return AP(blob16.tensor, b16o + r * 15, [[b16p, 2 * C], [1, 15]])

        def wconvB_ap(r):        # [48, 15] stationary: tap (r,2)
            return AP(blob16.tensor, b16o + 45 + r * 15, [[b16p, C], [1, 15]])

        def dwt_ap(ch, nparts):  # [nparts, 48] stationary for dw chunk ch
            return AP(blob16.tensor, b16o + 90 + ch * C, [[b16p, nparts], [1, C]])

        def b32col(col, nparts):
            return AP(blob32.tensor, b32o + col, [[b32p, nparts], [1, 1]])

        bconv_ap = b32col(126, 15)

        id128 = cp.tile([128, 128], dt.bfloat16, tag="id128")
        make_identity(nc, id128[:])
        id48 = id128[0:C, 0:C]
        id16 = id128[0:15, 0:15]

        # Dummy 128-descriptor gather issued first: forces the Pool engine's
        # gather-ucode LOAD_LIB (which barriers on all previously emitted
        # work) to happen immediately against an empty pipeline instead of
        # right before the first real gather.
        dz_idx = tp.tile([128, 8], dt.int16, tag="dz_idx")
        nc.gpsimd.memset(dz_idx[:], 0)
        dz_out = tp.tile([128, 256], dt.bfloat16, tag="dz_out")
        nc.gpsimd.dma_gather(
            out_ap=AP(dz_out.tensor, dz_out[:].offset,
                      [dz_out[:].ap[0], [QW, 1], [1, QW]]),
            in_ap=x_quad[:], idxs_ap=dz_idx[:],
            num_idxs=128, num_idxs_reg=128,
            elem_size=QW, single_packet=False)

        # ---------------- zero the quad-table edge rows ----------------
        # Rows [0, LEAD) and [RQ-127, RQ) can be fetched (clamped idx /
        # out-of-image corners) with zero weight; they must hold 0, not junk.
        ztile = tp.tile([128, 254], dt.bfloat16, tag="ztile")
        nc.vector.memset(ztile[:], 0.0)
        zlow = nc.sync.dma_start(
            out=AP(x_quad, 0, [[148, 128], [1, 148]]),
            in_=ztile[:, :148])
        zhigh = nc.sync.dma_start(
            out=AP(x_quad, (RQ - 127) * QW, [[254, 128], [1, 254]]),
            in_=ztile[:, :254])

        # ---------------- padded CHW copy (bf16) for convs ----------------
        # x_pad2 partitions 0..47 hold padded x; partitions 48..95 hold the
        # same shifted one column left (tap c=1), so one matmul covers two
        # taps with a [96, 15] stationary. Only pad cells are memset.
        x_pad2 = cp.tile([2 * C, PADLEN], dt.bfloat16, tag="x_pad2")
        xp_p = x_pad2[:].ap[0][0]
        xp_o = x_pad2[:].offset
        nc.vector.memset(x_pad2[0:C, 0:PADW + 1], 0.0)
        nc.vector.memset(
            AP(x_pad2.tensor, xp_o + 2 * PADW - 1,
               [[xp_p, C], [PADW, H - 1], [1, 2]]), 0.0)
        nc.vector.memset(x_pad2[0:C, (H + 1) * PADW - 1:PADLEN], 0.0)
        xpad_int = AP(x_pad2.tensor, xp_o + PADW + 1,
                      [[xp_p, C], [PADW, H], [1, W]])
        nc.vector.tensor_copy(xpad_int, x_sb[:])
        nc.sync.dma_start(
            out=AP(x_pad2.tensor, xp_o + C * xp_p,
                   [[xp_p, C], [1, PADLEN - 1]]),
            in_=AP(x_pad2.tensor, xp_o + 1, [[xp_p, C], [1, PADLEN - 1]]))

        # ---------------- x -> HWC (bf16) via PE transposes ----------------
        # One tile per table band so band writes only dep on their own chunks.
        hwc_tiles = [cp.tile([128, (k1 - k0) * C], dt.bfloat16,
                             name=f"x_hwc{i}", tag=f"x_hwc{i}")
                     for i, (k0, k1) in enumerate(BANDS)]

        def hwc_of(chunk):
            for (k0, k1), t in zip(BANDS, hwc_tiles):
                if k0 <= chunk < k1:
                    return t, k0
            raise AssertionError

        def emit_trans(g):                    # 4 blocks per PSUM tile
            nblk = min(4, NB - g * 4)
            ps = pp_a.tile([128, 4 * C], dt.bfloat16, name="psA", tag="psA")
            for j in range(nblk):
                b = g * 4 + j
                nc.tensor.transpose(ps[:, j * C:(j + 1) * C],
                                    x_sb[:, b * 128:(b + 1) * 128], id48[:])
            t, k0 = hwc_of(g * 4)
            nc.scalar.activation(
                t[:, (g * 4 - k0) * C:(g * 4 - k0 + nblk) * C],
                ps[:, :nblk * C], act.Copy)

        # ---------------- quad table: 4 slot DMAs per band -----------------
        band_ins = {}

        def emit_band(bi):
            k0, k1 = BANDS[bi]
            nk = k1 - k0
            t, tk0 = hwc_of(k0)
            ws = []
            for j, dj in enumerate(DELTA):
                base = (LEAD - dj) * QW + 64 * j
                w = nc.sync.dma_start(
                    out=AP(x_quad, base + k0 * 128 * QW,
                           [[QW, 128], [128 * QW, nk], [1, C]]),
                    in_=AP(t.tensor, t[:].offset + (k0 - tk0) * C,
                           [t[:].ap[0], [C, nk], [1, C]]))
                if bi == 0:
                    add_dep_helper(w.ins, zlow.ins, reason="slot after zero")
                if bi == 2:
                    add_dep_helper(w.ins, zhigh.ins, reason="slot after zero")
                ws.append(w)
            band_ins[bi] = ws

        def _slice_table_deps(b0, nb):
            y_lo = (b0 * 128) // W
            y_hi = ((b0 + nb) * 128 - 1) // W
            r_lo = max(0, LEAD + (y_lo - MAXD) * W)
            r_hi = min(RQ - 1, LEAD + (y_hi + MAXD + 1) * W + W - 1)
            deps = []
            if r_lo < LEAD:
                deps.append(zlow)
            if r_hi >= RQ - 127:
                deps.append(zhigh)
            for bi, (k0, k1) in enumerate(BANDS):
                if r_lo <= k1 * 128 + W + 1 and r_hi >= k0 * 128 + 1:
                    deps.extend(band_ins[bi])
            return deps

        # bn scale' = gamma * rsqrt(rvar+eps); shift' = beta - rmean*scale'
        def emit_bn():
            veps = tp.tile([C, 1], dt.float32, tag="veps")
            nc.vector.tensor_scalar(veps[:], b32col(130, C), BN_EPS, None, op.add)
            vsq = tp.tile([C, 1], dt.float32, tag="vsq")
            nc.scalar.activation(vsq[:], veps[:], act.Sqrt)
            vri = tp.tile([C, 1], dt.float32, tag="vri")
            nc.vector.reciprocal(vri[:], vsq[:])
            scale_t = cp.tile([C, 1], dt.float32, tag="scale")
            nc.vector.tensor_tensor(scale_t[:], b32col(127, C), vri[:], op.mult)
            vms = tp.tile([C, 1], dt.float32, tag="vms")
            nc.vector.tensor_tensor(vms[:], b32col(129, C), scale_t[:], op.mult)
            shift_t = cp.tile([C, 1], dt.float32, tag="shift")
            nc.vector.tensor_tensor(shift_t[:], b32col(128, C), vms[:], op.subtract)
            return scale_t, shift_t

        # ---------------- convs: 15ch 3x3 via 6 accumulated matmuls --------
        conv_sb = cp.tile([15, HW], dt.bfloat16, tag="conv_sb")
        trows = [(t * CONV_ROWS, min(CONV_ROWS, H - t * CONV_ROWS))
                 for t in range((H + CONV_ROWS - 1) // CONV_ROWS)]

        def conv_tile(r0, nr):
            psc = pp_a.tile([15, CONV_ROWS * W], dt.float32, name="psA2",
                            tag="psA2", bufs=3)
            npx = nr * W
            for r in range(3):
                rhsA = AP(x_pad2.tensor, xp_o + (r0 + r) * PADW,
                          [[xp_p, 2 * C], [PADW, nr], [1, W]])
                nc.tensor.matmul(psc[:, :npx], wconvA_ap(r), rhsA,
                                 start=(r == 0), stop=False)
                rhsB = AP(x_pad2.tensor, xp_o + (r0 + r) * PADW + 2,
                          [[xp_p, C], [PADW, nr], [1, W]])
                nc.tensor.matmul(psc[:, :npx], wconvB_ap(r), rhsB,
                                 start=False, stop=(r == 2))
            nc.scalar.activation(conv_sb[:, r0 * W:r0 * W + npx], psc[:, :npx],
                                 act.Identity, bias=bconv_ap)

        # conv output -> pixel-major (tcols), per 8-block group
        tcols = cp.tile([128, NB * 15], dt.float32, tag="tcols")

        def tcols_group(g):
            nblk = min(8, NB - g * 8)
            ps = pp_a.tile([128, 8 * 16], dt.bfloat16, name="psA", tag="psA")
            for j in range(nblk):
                b = g * 8 + j
                nc.tensor.transpose(ps[:, j * 16:j * 16 + 15],
                                    conv_sb[:, b * 128:(b + 1) * 128],
                                    id16[:])
            nc.scalar.activation(tcols[:, g * 8 * 15:(g * 8 + nblk) * 15],
                                 AP(ps.tensor, ps[:].offset,
                                    [ps[:].ap[0], [16, nblk], [1, 15]]),
                                 act.Copy)

        # ---------------- bulk offset / weight / index math ----------------
        def plane(tag):
            return tp.tile([128, QTOT], dt.float32, name=tag, tag=tag)

        py = plane("py"); px = plane("px")
        t1 = plane("t1"); t2 = plane("t2"); t3 = plane("t3")
        fy = plane("fy"); fx = plane("fx")
        y0 = plane("y0"); x0 = plane("x0")
        wy0 = py; wy1 = px                       # reuse dead slots (disjoint cols)
        wx0 = plane("wx0"); wx1 = plane("wx1")
        idxf = fy                                # reuse dead slot
        cvt_i = tp.tile([128, QTOT], dt.int32, name="cvt_i", tag="cvt_i")
        w4 = cp.tile([128, QTOT * 4], dt.bfloat16, tag="w4")
        idx_t = cp.tile([128, QTOT], dt.int16, tag="idx")
        fold = cp.tile([16, 8 * QTOT], dt.int16, tag="fold")
        # one idxw tile per offset-math group, so a slice gather depends only
        # on its own group's fold (Tile tracks whole-tile versions)
        idxw_tiles = {}
        group_repls = []
        groups = [(0, 4), (4, 4), (8, 8), (16, 16), (32, 22)]

        def emit_group(g0, gn):
            nq = gn * 9
            c0 = g0 * 9
            sl = slice(c0, c0 + nq)

            def tcol_b(ch):
                return AP(tcols.tensor, tcols[:].offset + ch + g0 * 15,
                          [tcols[:].ap[0], [15, gn], [0, 9]])

            def reg_b(roff):
                return AP(blob32.tensor, b32o + roff,
                          [[b32p, 128], [0, gn], [1, 9]])

            def coord_b(coff):
                return AP(blob32.tensor, b32o + coff + g0,
                          [[b32p, 128], [1, gn], [0, 9]])

            tt = nc.vector.tensor_tensor
            ts = nc.vector.tensor_scalar
            tt(t1[:, sl], reg_b(0), tcol_b(0), op.mult)
            tt(t2[:, sl], reg_b(9), tcol_b(1), op.mult)
            tt(t3[:, sl], t1[:, sl], t2[:, sl], op.add)
            tt(t1[:, sl], t3[:, sl], tcol_b(4), op.add)
            tt(py[:, sl], t1[:, sl], coord_b(18), op.add)
            tt(t1[:, sl], reg_b(0), tcol_b(2), op.mult)
            tt(t2[:, sl], reg_b(9), tcol_b(3), op.mult)
            tt(t3[:, sl], t1[:, sl], t2[:, sl], op.add)
            tt(t1[:, sl], t3[:, sl], tcol_b(5), op.add)
            tt(px[:, sl], t1[:, sl], coord_b(72), op.add)

            # floor(v) = int(v) - (v < int(v)); rounding-mode agnostic
            for (v, fl, fr) in ((py, y0, fy), (px, x0, fx)):
                nc.vector.tensor_copy(cvt_i[:, sl], v[:, sl])
                nc.vector.tensor_copy(t1[:, sl], cvt_i[:, sl])
                tt(t2[:, sl], v[:, sl], t1[:, sl], op.is_lt)
                tt(fl[:, sl], t1[:, sl], t2[:, sl], op.subtract)
                tt(fr[:, sl], v[:, sl], fl[:, sl], op.subtract)

            mk_b = AP(tcols.tensor, tcols[:].offset + 6 + g0 * 15,
                      [tcols[:].ap[0], [15, gn], [1, 9]])
            ts(t1[:, sl], y0[:, sl], 0.0, None, op.is_ge)
            ts(t2[:, sl], y0[:, sl], float(H - 1), None, op.is_le)
            tt(t3[:, sl], t1[:, sl], t2[:, sl], op.mult)
            ts(t1[:, sl], fy[:, sl], -1.0, 1.0, op.mult, op.add)
            tt(wy0[:, sl], t1[:, sl], t3[:, sl], op.mult)
            ts(t1[:, sl], y0[:, sl], -1.0, None, op.is_ge)
            ts(t2[:, sl], y0[:, sl], float(H - 2), None, op.is_le)
            tt(t3[:, sl], t1[:, sl], t2[:, sl], op.mult)
            tt(wy1[:, sl], fy[:, sl], t3[:, sl], op.mult)
            ts(t1[:, sl], x0[:, sl], 0.0, None, op.is_ge)
            ts(t2[:, sl], x0[:, sl], float(W - 1), None, op.is_le)
            tt(t3[:, sl], t1[:, sl], t2[:, sl], op.mult)
            ts(t1[:, sl], fx[:, sl], -1.0, 1.0, op.mult, op.add)
            tt(wx0[:, sl], t1[:, sl], t3[:, sl], op.mult)
            ts(t1[:, sl], x0[:, sl], -1.0, None, op.is_ge)
            ts(t2[:, sl], x0[:, sl], float(W - 2), None, op.is_le)
            tt(t3[:, sl], t1[:, sl], t2[:, sl], op.mult)
            tt(wx1[:, sl], fx[:, sl], t3[:, sl], op.mult)
            tt(wy0[:, sl], wy0[:, sl], mk_b, op.mult)
            tt(wy1[:, sl], wy1[:, sl], mk_b, op.mult)

            def w4_slot(j):
                return AP(w4.tensor, w4[:].offset + j + c0 * 4,
                          [w4[:].ap[0], [4, nq]])
            tt(w4_slot(0), wy0[:, sl], wx0[:, sl], op.mult)
            tt(w4_slot(1), wy0[:, sl], wx1[:, sl], op.mult)
            tt(w4_slot(2), wy1[:, sl], wx0[:, sl], op.mult)
            tt(w4_slot(3), wy1[:, sl], wx1[:, sl], op.mult)

            # gather index = table row r = clip(LEAD + y0*W + x0, 0, RQ-1)
            nc.vector.scalar_tensor_tensor(idxf[:, sl], y0[:, sl], float(W),
                                           x0[:, sl], op.mult, op.add)
            ts(idxf[:, sl], idxf[:, sl], float(LEAD), 0.0, op.add, op.max)
            ts(idxf[:, sl], idxf[:, sl], float(RQ - 1), None, op.min)
            nc.vector.tensor_copy(idx_t[:, sl], idxf[:, sl])

            # 16-wrap via DRAM round-trip fold + in-partition interleave;
            # queue 0 reads idx partitions 0..31 only -> replicate once.
            wr = nc.sync.dma_start(
                out=AP(idx_dram, c0, [[QTOT, 128], [1, nq]]),
                in_=idx_t[:, sl])
            rdf = nc.sync.dma_start(
                out=AP(fold.tensor, fold[:].offset + c0,
                       [fold[:].ap[0], [QTOT, 8], [1, nq]]),
                in_=AP(idx_dram, c0, [[QTOT, 16], [16 * QTOT, 8], [1, nq]]))
            add_dep_helper(rdf.ins, wr.ins, sync=False, reason="idx fold rt")
            idxw = cp.tile([128, nq * 8], dt.int16, tag=f"idxw{g0}")
            idxw_tiles[g0] = (g0, gn, idxw)
            i16 = idxw[0:16, :]
            nc.vector.tensor_copy(
                AP(i16.tensor, i16.offset,
                   [i16.ap[0], [1, 8], [8, nq]]),
                AP(fold.tensor, fold[:].offset + c0,
                   [fold[:].ap[0], [QTOT, 8], [1, nq]]))
            nc.sync.dma_start(out=idxw[16:32, :], in_=idxw[0:16, :])
            nc.sync.dma_start(out=idxw[32:64, :], in_=idxw[0:32, :])
            r3 = nc.sync.dma_start(out=idxw[64:128, :], in_=idxw[0:64, :])
            group_repls.append(r3)

        # ---------------- emission schedule ----------------
        # All offset-math groups complete BEFORE the first gather launches:
        # DVE ops that execute while a Pool gather is in flight can stall
        # until the gather retires (observed on HW), so the gather window is
        # kept to combine/epilogue ops that are proven immune. PE runs the
        # convs first (they feed tcols -> group math), then the x_hwc
        # transposes that feed the table-band writes.
        conv_tile(*trows[0]); conv_tile(*trows[1]); conv_tile(*trows[2])
        tcols_group(0)
        emit_group(0, 4)
        emit_group(4, 4)
        scale_t, shift_t = emit_bn()
        conv_tile(*trows[3]); conv_tile(*trows[4])
        tcols_group(1)
        emit_group(8, 8)
        for t_ in range(5, 9):
            conv_tile(*trows[t_])
        tcols_group(2); tcols_group(3)
        emit_group(16, 16)
        for t_ in range(9, 14):
            conv_tile(*trows[t_])
        for g in range(4, 7):
            tcols_group(g)
        emit_group(32, 22)
        for g in range(14):
            emit_trans(g)
        emit_band(0); emit_band(1); emit_band(2)

        # ---------------- gather / combine / matmul / epilogue -------------
        for si, (b0, nb) in enumerate(_slices()):
            Q = nb * 9
            npx = nb * 128
            q0 = b0 * 9
            gq = wp.tile([128, SLICE_BLOCKS * 9 * QW], dt.bfloat16, tag="gq",
                         bufs=4)
            gq3 = AP(gq.tensor, gq[:].offset,
                     [gq[:].ap[0], [QW, Q], [1, QW]])
            grp = next(v for v in idxw_tiles.values()
                       if v[0] <= b0 and b0 + nb <= v[0] + v[1])
            lq = (b0 - grp[0]) * 9
            gi = nc.gpsimd.dma_gather(
                out_ap=gq3,
                in_ap=x_quad[:],
                idxs_ap=grp[2][:, lq * 8:(lq + Q) * 8],
                num_idxs=Q * 128,
                num_idxs_reg=Q * 128,
                elem_size=QW,
                single_packet=False,
                queue_num=si % 4,
            )
            for dep in _slice_table_deps(b0, nb):
                add_dep_helper(gi.ins, dep.ins, reason="gather after table")
            if si == 0:
                for dep in group_repls:
                    add_dep_helper(gi.ins, dep.ins,
                                   reason="gathers after all group math")

            # weighted 4-corner combine (in-place over the gathered tile;
            # only the 48 valid channels of each 64-wide slot are touched)
            prod = gq
            gview = AP(gq.tensor, gq[:].offset,
                       [gq[:].ap[0], [QW, Q], [64, 4], [1, C]])
            wb = AP(w4.tensor, w4[:].offset + q0 * 4,
                    [w4[:].ap[0], [4, Q], [1, 4], [0, C]])
            nc.vector.tensor_tensor(gview, gview, wb, op.mult)

            def pview(off):
                return AP(prod.tensor, prod[:].offset + off,
                          [prod[:].ap[0], [QW, Q], [1, C]])
            sa = wp.tile([128, SLICE_BLOCKS * 9 * C], dt.bfloat16, tag="sa")
            # sampt doubles as scratch for the second corner-pair sum; its
            # cols are rewritten only after the final add consumed them
            sampt = wp.tile([128, 4 * SLICE_BLOCKS * 128], dt.bfloat16, tag="sampt")
            sb2 = sampt
            nc.vector.tensor_tensor(sa[:, :Q * C], pview(0), pview(64), op.add)
            nc.vector.tensor_tensor(sb2[:, :Q * C], pview(128), pview(192),
                                    op.add)
            samp = sa
            nc.vector.tensor_tensor(samp[:, :Q * C], sa[:, :Q * C],
                                    sb2[:, :Q * C], op.add)
            for ib in range(nb):
                ps = pp_st.tile([128, 512], dt.bfloat16, tag="ps_st")
                base = ib * 9 * C
                for ch in range(3):
                    nc.tensor.transpose(
                        ps[:, ch * 128:(ch + 1) * 128],
                        samp[:, base + ch * 128: base + (ch + 1) * 128],
                        id128[:])
                nc.tensor.transpose(ps[0:C, 384:512],
                                    samp[:, base + 384: base + 432], id128[:])
                dst02 = AP(sampt.tensor, sampt[:].offset + ib * 128,
                           [sampt[:].ap[0], [npx, 3], [1, 128]])
                nc.scalar.activation(dst02, ps[:, 0:384], act.Copy)
                nc.scalar.activation(sampt[0:C, 3 * npx + ib * 128:
                                           3 * npx + (ib + 1) * 128],
                                     ps[0:C, 384:512], act.Copy)

            # matmul: out[o, px] += dwT_chunk.T @ sampt_chunk
            pso = pp_out.tile([C, SLICE_BLOCKS * 128], dt.float32, tag="ps_out")
            for ch in range(3):
                nc.tensor.matmul(pso[:, :npx], dwt_ap(ch, 128),
                                 sampt[:, ch * npx:(ch + 1) * npx],
                                 start=(ch == 0), stop=False)
            nc.tensor.matmul(pso[:, :npx], dwt_ap(3, C),
                             sampt[0:C, 3 * npx:3 * npx + npx],
                             start=False, stop=True)

            # epilogue: BN(running stats) + residual (DVE) + relu (ACT)
            bno = wp.tile([C, SLICE_BLOCKS * 128], dt.float32, tag="bno")
            nc.scalar.activation(bno[:, :npx], pso[:, :npx], act.Identity,
                                 bias=shift_t[:], scale=scale_t[:])
            nc.vector.tensor_tensor(bno[:, :npx], bno[:, :npx],
                                    x_sb[:, b0 * 128:b0 * 128 + npx], op.add)
            bno2 = wp.tile([C, SLICE_BLOCKS * 128], dt.float32, tag="bno2")
            nc.scalar.activation(bno2[:, :npx], bno[:, :npx], act.Relu)
            nc.sync.dma_start(out_ext[:, b0 * 128:b0 * 128 + npx],
                              bno2[:, :npx])

    nc.compile()
    return nc


def _host_pack(inputs):
    """Weight/constant layout prep (no input-data compute)."""
    bf16 = ml_dtypes.bfloat16
    wa = np.concatenate([inputs['tm_w'], inputs['tr_w'], inputs['mk_w']],
                        axis=0)                       # [15, C, 3, 3]
    bconv = np.concatenate([inputs['tm_b'], inputs['tr_b'],
                            inputs['mk_b']]).astype(np.float32)
    dwr = inputs['dw'].reshape(C, C, 9)               # [o, c, k]
    dwT = np.zeros((512, C), dtype=np.float32)
    dwT[:432] = dwr.transpose(2, 1, 0).reshape(9 * C, C)
    dwt = np.ascontiguousarray(
        dwT.reshape(4, 128, C).transpose(1, 0, 2).reshape(128, 4 * C)
    ).astype(bf16)

    blob16 = np.zeros((128, 282), dtype=bf16)
    for r in range(3):
        blob16[:C, r * 15:(r + 1) * 15] = wa[:, :, r, 0].T
        blob16[C:2 * C, r * 15:(r + 1) * 15] = wa[:, :, r, 1].T
        blob16[:C, 45 + r * 15:45 + (r + 1) * 15] = wa[:, :, r, 2].T
    blob16[:, 90:282] = dwt

    pix = (np.arange(NB)[None, :] * 128 + np.arange(128)[:, None])
    blob32 = np.zeros((128, 131), dtype=np.float32)
    blob32[:, 0:9] = np.tile(_REG[0], (128, 1))
    blob32[:, 9:18] = np.tile(_REG[1], (128, 1))
    blob32[:, 18:72] = (pix // W).astype(np.float32)
    blob32[:, 72:126] = (pix % W).astype(np.float32)
    blob32[:15, 126] = bconv
    blob32[:C, 127] = inputs['gamma'].astype(np.float32)
    blob32[:C, 128] = inputs['beta'].astype(np.float32)
    blob32[:C, 129] = inputs['rmean'].astype(np.float32)
    blob32[:C, 130] = inputs['rvar'].astype(np.float32)
    return dict(blob16=blob16, blob32=blob32)


def kernel(**inputs):
    inputs = {k: np.asarray(v) for k, v in inputs.items()}
    if 'nc' not in _built:
        _built['nc'] = build_nc()
    nc = _built['nc']

    from concourse.bass_utils import run_bass_kernel_spmd
    shared = _host_pack(inputs)
    x = inputs['x'].astype(ml_dtypes.bfloat16)
    in_maps = []
    for i in range(N):
        m = dict(shared)
        m['x'] = np.ascontiguousarray(x[i].reshape(C, HW))
        in_maps.append(m)
    res = run_bass_kernel_spmd(nc, in_maps, core_ids=list(range(N)))
    out = np.stack([res.results[i]['out'].reshape(C, H, W)
                    for i in range(N)])
    return out.astype(np.float32)
